# revision 1
# baseline (speedup 1.0000x reference)
"""Trainium2 Bass kernel for nn_Autoencoder_65223373357102 (FLAME-style autoencoder).

Strategy:
  Phase 1 (8-way tensor parallel): encoder GEMM [64,150528]@[150528,556] sharded
  along the input-feature axis. Each core transposes its x shard on TensorE,
  multiplies against its 1/8 slice of enc_W, adds enc_b/8 via a K=1 matmul, and
  AllReduces the [64,556] latent (142 KB).
  Phase 2 (replicated): blendshape GEMM [64,400]@[400,3*5023] in plane-separated
  layout + all per-batch geometry with batch on partitions; per-batch scalars are
  broadcast along the free axis via tensor_scalar. Every core computes the full
  output; the host takes core 0's copy.
"""
import sys
import types

sys.path.insert(0, "/opt/trn_rl_repo")

import numpy as np


def _ensure_ntff_hook():
    """Provide antenv.axon_hooks + install the ctypes NTFF profile hook so
    run_bass_kernel_spmd(trace=True) can pull a neuron-profile under axon."""
    name = "antenv.axon_hooks"
    if name not in sys.modules:
        mod = types.ModuleType(name)
        mod._HOOK = None

        def set_axon_ntff_profile_hook(hook):
            mod._HOOK = hook

        def get_axon_ntff_profile_hook():
            return mod._HOOK

        mod.set_axon_ntff_profile_hook = set_axon_ntff_profile_hook
        mod.get_axon_ntff_profile_hook = get_axon_ntff_profile_hook
        sys.modules[name] = mod
        try:
            import antenv

            antenv.axon_hooks = mod
        except ImportError:
            pass
    mod = sys.modules[name]
    if mod.get_axon_ntff_profile_hook() is None:
        try:
            from trn_agent_boot.trn_boot import _ntff_profile_via_ctypes

            hook = _ntff_profile_via_ctypes("/opt/axon/libaxon_pjrt.so")
            if hook is not None:
                mod.set_axon_ntff_profile_hook(hook)
        except Exception:
            pass


_ensure_ntff_hook()

from concourse import bass, mybir, tile
from concourse.bass_utils import run_bass_kernel_spmd

F32 = mybir.dt.float32
ALU = mybir.AluOpType
ACTF = mybir.ActivationFunctionType
AX = mybir.AxisListType

B = 64
V = 5023
VM = 3500
LAT = 556
DIN = 3 * 224 * 224  # 150528
NCORES = 8
KSH = DIN // NCORES  # 18816
KTILES = KSH // 128  # 147
NOUT = 2 * VM + 68 + 11  # 7079
GAZE_DIR = -1.0
HALF_PI = 1.5707963267948966


def _chunks(total, step):
    out = []
    o = 0
    while o < total:
        out.append((o, min(step, total - o)))
        o += step
    return out


class Geo:
    """Helper for tiny per-batch scalar ops on [rows,1] tiles."""

    _uid = [0]

    def __init__(self, nc, pool, rows=B):
        self.nc = nc
        self.pool = pool
        self.rows = rows

    def t(self, cols=1):
        Geo._uid[0] += 1
        return self.pool.tile([self.rows, cols], F32, name=f"g{Geo._uid[0]}_{cols}")

    def mul(self, a, b):
        o = self.t()
        self.nc.vector.tensor_tensor(out=o, in0=a, in1=b, op=ALU.mult)
        return o

    def add(self, a, b):
        o = self.t()
        self.nc.vector.tensor_tensor(out=o, in0=a, in1=b, op=ALU.add)
        return o

    def sub(self, a, b):
        o = self.t()
        self.nc.vector.tensor_tensor(out=o, in0=a, in1=b, op=ALU.subtract)
        return o

    def mac(self, a, s, acc):
        """(a * s) + acc, s is a [B,1] AP scalar."""
        o = self.t()
        self.nc.vector.scalar_tensor_tensor(
            out=o, in0=a, scalar=s, in1=acc, op0=ALU.mult, op1=ALU.add
        )
        return o

    def dot3(self, ax, ay, az, bx, by, bz):
        o = self.mul(ax, bx)
        o = self.mac(ay, by, o)
        o = self.mac(az, bz, o)
        return o

    def cross3(self, ax, ay, az, bx, by, bz):
        """a x b -> 3 [B,1] tiles."""
        cx = self.sub(self.mul(ay, bz), self.mul(az, by))
        cy = self.sub(self.mul(az, bx), self.mul(ax, bz))
        cz = self.sub(self.mul(ax, by), self.mul(ay, bx))
        return cx, cy, cz


def axis_angle_R(nc, g, aa3, pfx, halfpi):
    R_ = g.rows
    """aa3: [B,3] axis-angle tile -> R [B,9] tile, R[l,i] at col l*3+i.

    R = c*I + s*K + (1-c) a a^T  (Rodrigues, matching reference)
    """
    pool = g.pool
    sq = pool.tile([R_, 3], F32, name=pfx + "aaR_sq")
    nc.vector.tensor_tensor(out=sq, in0=aa3, in1=aa3, op=ALU.mult)
    th2 = g.t()
    nc.vector.tensor_reduce(out=th2, in_=sq, axis=AX.X, op=ALU.add)
    theta = g.t()
    nc.scalar.activation(out=theta, in_=th2, func=ACTF.Sqrt)
    thm = g.t()
    nc.vector.tensor_scalar_max(out=thm, in0=theta, scalar1=1e-8)
    rth = g.t()
    nc.vector.reciprocal(out=rth, in_=thm)
    axis3 = pool.tile([R_, 3], F32, name=pfx + "aaR_axis")
    nc.vector.tensor_scalar_mul(out=axis3, in0=aa3, scalar1=rth)
    s = g.t()
    nc.scalar.activation(out=s, in_=theta, func=ACTF.Sin)
    c = g.t()
    nc.scalar.activation(out=c, in_=theta, func=ACTF.Sin, bias=halfpi)
    omc = g.t()
    nc.vector.tensor_scalar(
        out=omc, in0=c, scalar1=-1.0, scalar2=1.0, op0=ALU.mult, op1=ALU.add
    )
    ax, ay, az = axis3[:, 0:1], axis3[:, 1:2], axis3[:, 2:3]
    # diag: omc*a_i^2 + c
    asq = pool.tile([R_, 3], F32, name=pfx + "aaR_asq")
    nc.vector.tensor_tensor(out=asq, in0=axis3, in1=axis3, op=ALU.mult)
    R = pool.tile([R_, 9], F32, name=pfx + "aaR_R")
    dmul = pool.tile([R_, 3], F32, name=pfx + "aaR_dmul")
    nc.vector.tensor_scalar_mul(out=dmul, in0=asq, scalar1=omc)
    # s*a
    sa = pool.tile([R_, 3], F32, name=pfx + "aaR_sa")
    nc.vector.tensor_scalar_mul(out=sa, in0=axis3, scalar1=s)
    sax, say, saz = sa[:, 0:1], sa[:, 1:2], sa[:, 2:3]
    # off-diag products omc*ai*aj
    mxy = g.mul(g.mul(ax, ay), omc)
    mxz = g.mul(g.mul(ax, az), omc)
    myz = g.mul(g.mul(ay, az), omc)
    # assemble diag: R[l*4] = dmul_l + c
    for l in range(3):
        nc.vector.tensor_tensor(
            out=R[:, 4 * l:4 * l + 1], in0=dmul[:, l:l + 1], in1=c, op=ALU.add
        )
    nc.vector.tensor_tensor(out=R[:, 1:2], in0=mxy, in1=saz, op=ALU.subtract)  # R01
    nc.vector.tensor_tensor(out=R[:, 2:3], in0=mxz, in1=say, op=ALU.add)  # R02
    nc.vector.tensor_tensor(out=R[:, 3:4], in0=mxy, in1=saz, op=ALU.add)  # R10
    nc.vector.tensor_tensor(out=R[:, 5:6], in0=myz, in1=sax, op=ALU.subtract)  # R12
    nc.vector.tensor_tensor(out=R[:, 6:7], in0=mxz, in1=say, op=ALU.subtract)  # R20
    nc.vector.tensor_tensor(out=R[:, 7:8], in0=myz, in1=sax, op=ALU.add)  # R21
    return R


_ENG_ATTR = {
    "SP": "sync", "Pool": "gpsimd", "PE": "tensor",
    "DVE": "vector", "Activation": "scalar",
}


def _legalize_waits(nc):
    """This walrus accepts only one sync-wait slot per instruction; move extra
    waits onto same-engine NoOps inserted right before the instruction."""
    import concourse.mybir as _mybir

    def make_nop(engine):
        eng = getattr(nc, _ENG_ATTR[engine.name])
        bi = eng.nop(nofuse=True)
        mi = bi.ins
        for bb in nc.main_func.blocks:
            if bb.instructions and bb.instructions[-1].name == mi.name:
                bb.instructions.pop()
                break
        mi.engine = engine
        return mi

    for bb in nc.main_func.blocks:
        snapshot = list(bb.instructions)
        newlist = []
        changed = False
        for inst in snapshot:
            si = inst.sync_info
            waits = list(si.on_wait) if (si and si.on_wait) else []
            if (
                len(waits) > 1
                and not inst.name.startswith("barrier")
                and inst.engine is not None
                and getattr(inst.engine, "name", None) in _ENG_ATTR
            ):
                for w in waits[:-1]:
                    nop = make_nop(inst.engine)
                    nop.sync_info = _mybir.SyncInfo(on_wait=[w], on_update=[])
                    newlist.append(nop)
                inst.sync_info = _mybir.SyncInfo(
                    on_wait=[waits[-1]], on_update=list(si.on_update)
                )
                changed = True
            newlist.append(inst)
        if changed:
            bb.instructions[:] = newlist


def build_graph(fl_idx, idx4, idx2, l_lo, r_lo):
    """fl_idx: 68 ints (vert cols for masked landmarks), idx4/idx2: landmark vert
    cols, l_lo/r_lo: start of the contiguous eye ranges."""
    nc = bass.Bass(target_bir_lowering=False)

    x_p = nc.declare_dram_parameter("x_sh", [KSH, B], F32, isOutput=False)
    w_p = nc.declare_dram_parameter("w_sh", [KSH, LAT], F32, isOutput=False)
    b_p = nc.declare_dram_parameter("enc_b", [1, LAT + 128 + 3], F32, isOutput=False)
    bm_p = nc.declare_dram_parameter("bmean", [128, 12], F32, isOutput=False)
    tpl_p = nc.declare_dram_parameter("tmpl", [3, V], F32, isOutput=False)
    bas_p = nc.declare_dram_parameter("basis", [400, 3, V], F32, isOutput=False)
    cam_p = nc.declare_dram_parameter("cam", [B, 12], F32, isOutput=False)
    out_p = nc.declare_dram_parameter("out", [B, 3, NOUT], F32, isOutput=True)

    ar_in = nc.dram_tensor("ar_in", [B, LAT], F32)
    ar_out = nc.dram_tensor("ar_out", [B, LAT], F32, addr_space="Shared")

    with tile.TileContext(nc) as tc:
        with (
            tc.tile_pool(name="consts", bufs=1) as consts,
            tc.tile_pool(name="latents", bufs=1) as latp,
            tc.tile_pool(name="geo", bufs=1) as geop,
            tc.tile_pool(name="planes", bufs=1) as planep,
            tc.tile_pool(name="dum", bufs=1, space="PSUM") as dum,
        ):
            b_sb = consts.tile([1, LAT + 128 + 3], F32)
            nc.sync.dma_start(out=b_sb, in_=b_p[:, :])
            ones8 = b_sb[:, LAT:LAT + B]       # value 1/NCORES, packed by host
            ones1 = b_sb[:, LAT + B:LAT + 2 * B]  # value 1.0, packed by host
            halfpi = consts.tile([128, 1], F32)
            nc.vector.memset(halfpi, HALF_PI)
            # PE matmuls carry a single sync-wait slot on this walrus; dummy
            # 1-wait matmuls make PE observe one dep before the real matmul.
            d1 = dum.tile([1, 1], F32)
            d64 = dum.tile([B, 1], F32)

            # ---------------- Phase 1: encoder GEMM ----------------
            NSPL = [(0, 512), (512, 44)]
            TPC = 7  # k-tiles per x chunk
            with (
                tc.tile_pool(name="xin", bufs=3) as xin,
                tc.tile_pool(name="wts", bufs=3) as wts,
                tc.tile_pool(name="encp", bufs=1, space="PSUM") as encp,
            ):
                pe = [encp.tile([B, n], F32, name=f"pe{j}", tag=f"pe{j}") for j, (_, n) in enumerate(NSPL)]
                x_view = x_p.ap().rearrange("(c t p) m -> c p t m", t=TPC, p=128)
                w_view = w_p.ap().rearrange("(c t p) m -> c p t m", t=TPC, p=128)
                for ci in range(KTILES // TPC):
                    x_c = xin.tile([128, TPC, B], F32)
                    nc.gpsimd.dma_start(out=x_c, in_=x_view[ci])
                    nc.tensor.matmul(
                        d1, lhsT=x_c[:, 0, 0:1], rhs=x_c[:, 0, 0:1],
                        start=True, stop=True, skip_group_check=True,
                    )
                    w_c = wts.tile([128, TPC, LAT], F32)
                    nc.sync.dma_start(out=w_c, in_=w_view[ci])
                    for t in range(TPC):
                        k = ci * TPC + t
                        for j, (n0, n) in enumerate(NSPL):
                            nc.tensor.matmul(
                                pe[j],
                                lhsT=x_c[:, t, :],
                                rhs=w_c[:, t, n0:n0 + n],
                                start=(k == 0),
                                stop=False,
                            )
                for j, (n0, n) in enumerate(NSPL):
                    nc.tensor.matmul(
                        pe[j],
                        lhsT=ones8,
                        rhs=b_sb[:, n0:n0 + n],
                        start=False,
                        stop=True,
                    )
                lat1 = latp.tile([B, LAT], F32)
                for j, (n0, n) in enumerate(NSPL):
                    nc.vector.tensor_copy(out=lat1[:, n0:n0 + n], in_=pe[j])
                nc.sync.dma_start(out=ar_in[:, :], in_=lat1)

            # prefetch the first basis chunks before the collective so the
            # DMA engines stay busy through the AllReduce bubble
            basp_ctx = tc.tile_pool(name="bas", bufs=12)
            basp = basp_ctx.__enter__()
            KSPL = [(0, 128, 128), (128, 128, 128), (256, 128, 128), (384, 16, 32)]
            VCH = _chunks(V, 512)
            bts = {}
            for j in (0, 1, 2):
                n0, n = VCH[j]
                for ki, (k0, kw, _cwa) in enumerate(KSPL):
                    bt = basp.tile([128, 3, 512], F32, name=f"btp{j}_{ki}", tag="bt")
                    nc.gpsimd.dma_start(
                        out=bt[:kw, :, :n], in_=bas_p[k0:k0 + kw, :, n0:n0 + n]
                    )
                    bts[(j, ki)] = bt
            nc.gpsimd.collective_compute(
                "AllReduce",
                ALU.add,
                replica_groups=[list(range(NCORES))],
                ins=[ar_in.ap().opt()],
                outs=[ar_out.ap().opt()],
            )
            lat = latp.tile([B, LAT], F32)
            nc.sync.dma_start(out=lat, in_=ar_out[:, :])

            # ---------------- Phase 1.5: transpose shape params ----------------
            # DVE 32x32 block transposes: spT[ki][r, b] = lat[b, c0+r].
            spT = []
            for (c0, kw, cwa) in KSPL:
                st = latp.tile([cwa, B], F32, name=f"spT{c0}", tag=f"spT{c0}")
                for pb in range(cwa // 32):
                    for fb in range(B // 32):
                        nc.vector.transpose(
                            out=st[32 * pb:32 * pb + 32, 32 * fb:32 * fb + 32],
                            in_=lat[32 * fb:32 * fb + 32,
                                    c0 + 32 * pb:c0 + 32 * pb + 32],
                        )
                spT.append(st)
            nc.tensor.matmul(
                d64, lhsT=spT[3], rhs=spT[3][:, 0:1],
                start=True, stop=True, skip_group_check=True,
            )

            # ---------------- Phase 2: blendshape + fused face transform ----------
            g = Geo(nc, geop)
            # vmean directly from latent: vm = tmpl_mean + shape_p @ basis_mean
            bm_sb = consts.tile([128, 12], F32)
            nc.sync.dma_start(out=bm_sb, in_=bm_p[:, :])
            with tc.tile_pool(name="vmp", bufs=1, space="PSUM") as vmp:
                pvm = vmp.tile([B, 3], F32)
                for ki, (k0, kw, _cwa) in enumerate(KSPL):
                    nc.tensor.matmul(
                        pvm, lhsT=spT[ki][:kw, :], rhs=bm_sb[:kw, ki * 3:ki * 3 + 3],
                        start=(ki == 0), stop=False,
                    )
                nc.tensor.matmul(
                    pvm, lhsT=ones1, rhs=b_sb[:, LAT + 128:LAT + 131],
                    start=False, stop=True,
                )
                vms = geop.tile([B, 3], F32)
                nc.vector.tensor_copy(out=vms, in_=pvm)

            # face rotation matrix, scaled
            aa_face = lat[:, 545:548]
            Rf = axis_angle_R(nc, g, aa_face, "f_", halfpi[:B, :])
            fs = g.t()  # face_scale = latent[551]+1
            nc.vector.tensor_scalar_add(out=fs, in0=lat[:, 551:552], scalar1=1.0)
            Rs = geop.tile([B, 9], F32)
            nc.vector.tensor_scalar_mul(out=Rs, in0=Rf, scalar1=fs)
            # offsets: off_i = face_t_i - sum_l vms_l*Rs[l,i]
            off = geop.tile([B, 3], F32)
            for i in range(3):
                t = g.mul(vms[:, 0:1], Rs[:, i:i + 1])
                t = g.mac(vms[:, 1:2], Rs[:, 3 + i:4 + i], t)
                t = g.mac(vms[:, 2:3], Rs[:, 6 + i:7 + i], t)
                nc.vector.tensor_tensor(
                    out=off[:, i:i + 1], in0=lat[:, 548 + i:549 + i], in1=t,
                    op=ALU.subtract,
                )

            # blendshape chunks; rotation fused per chunk into rt
            rt = planep.tile([B, 3, V], F32)
            with (
                tc.tile_pool(name="tpl", bufs=2) as tplp,
                tc.tile_pool(name="vstage", bufs=3) as vstp,
                tc.tile_pool(name="bpsum", bufs=3, space="PSUM") as bpsum,
            ):

                prev = []  # vstage read-APs for WAR-absorbing dummies
                for j, (n0, n) in enumerate(VCH):
                    vs = vstp.tile([B, 3, 512], F32)
                    if (j, 0) not in bts:
                        for ki, (k0, kw, _cwa) in enumerate(KSPL):
                            bt = basp.tile([128, 3, 512], F32, name=f"btl{j}_{ki}", tag="bt")
                            nc.gpsimd.dma_start(
                                out=bt[:kw, :, :n], in_=bas_p[k0:k0 + kw, :, n0:n0 + n]
                            )
                            bts[(j, ki)] = bt
                    for p in range(3):
                        gi = j * 3 + p
                        if gi >= 3:
                            pap = prev[gi - 3]
                            nc.tensor.matmul(
                                d1, lhsT=pap, rhs=pap,
                                start=True, stop=True, skip_group_check=True,
                            )
                        pv = bpsum.tile([B, 512], F32)
                        for ki, (k0, kw, _cwa) in enumerate(KSPL):
                            nc.tensor.matmul(
                                pv[:, :n],
                                lhsT=spT[ki][:kw, :],
                                rhs=bts[(j, ki)][:kw, p, :n],
                                start=(ki == 0),
                                stop=False,
                            )
                        tl = tplp.tile([1, 512], F32)
                        nc.sync.dma_start(out=tl[:, :n], in_=tpl_p[p:p + 1, n0:n0 + n])
                        nc.tensor.matmul(
                            pv[:, :n], lhsT=ones1, rhs=tl[:, :n],
                            start=False, stop=True,
                        )
                        nc.scalar.copy(out=vs[:, p, :n], in_=pv[:, :n])
                        prev.append(vs[:, p, 0:1])
                    for i in range(3):
                        nc.vector.tensor_scalar(
                            out=rt[:, i, n0:n0 + n], in0=vs[:, 0, :n],
                            scalar1=Rs[:, i:i + 1], scalar2=off[:, i:i + 1],
                            op0=ALU.mult, op1=ALU.add,
                        )
                        for l in (1, 2):
                            nc.vector.scalar_tensor_tensor(
                                out=rt[:, i, n0:n0 + n], in0=vs[:, l, :n],
                                scalar=Rs[:, 3 * l + i:3 * l + i + 1],
                                in1=rt[:, i, n0:n0 + n],
                                op0=ALU.mult, op1=ALU.add,
                            )
            basp_ctx.__exit__(None, None, None)

            # eye processing: both eyes stacked on 128 partitions
            # (rows 0:64 = left batch, 64:128 = right batch)
            EW = 546
            g2 = Geo(nc, geop, rows=128)
            es = geop.tile([128, 3, EW], F32)
            for i in range(3):
                nc.vector.tensor_copy(out=es[0:B, i, :], in_=rt[:, i, l_lo:l_lo + EW])
            nc.sync.dma_start(out=es[B:128, :, :], in_=rt[:, :, r_lo:r_lo + EW])
            # centers (mean over eye verts), both eyes at once
            cc = geop.tile([128, 3], F32)
            for i in range(3):
                nc.vector.tensor_reduce(
                    out=cc[:, i:i + 1], in_=es[:, i, :], axis=AX.X, op=ALU.add
                )
            c3 = geop.tile([128, 3], F32)
            nc.vector.tensor_scalar_mul(out=c3, in0=cc, scalar1=1.0 / EW)
            # pivot verts (l:4051, r:4597)
            pvt = geop.tile([128, 3, 1], F32)
            for i in range(3):
                nc.vector.tensor_copy(out=pvt[0:B, i, :], in_=rt[:, i, 4051:4052])
            nc.sync.dma_start(out=pvt[B:128, :, :], in_=rt[:, :, 4597:4598])
            # a = normalize(pivot - centre)
            a3 = geop.tile([128, 3], F32)
            for i in range(3):
                nc.vector.tensor_tensor(
                    out=a3[:, i:i + 1], in0=pvt[:, i, 0:1], in1=c3[:, i:i + 1],
                    op=ALU.subtract,
                )
            sqe = geop.tile([128, 3], F32)
            nc.vector.tensor_tensor(out=sqe, in0=a3, in1=a3, op=ALU.mult)
            n2 = g2.t()
            nc.vector.tensor_reduce(out=n2, in_=sqe, axis=AX.X, op=ALU.add)
            nn = g2.t()
            nc.scalar.activation(out=nn, in_=n2, func=ACTF.Sqrt)
            rn = g2.t()
            nc.vector.reciprocal(out=rn, in_=nn)
            nc.vector.tensor_scalar_mul(out=a3, in0=a3, scalar1=rn)
            ax, ay, az = a3[:, 0:1], a3[:, 1:2], a3[:, 2:3]
            # find_gaze_R: b=(0,0,GAZE_DIR); v = a x b = (ay*g, -ax*g, 0)
            vx = g2.t()
            nc.vector.tensor_scalar_mul(out=vx, in0=ay, scalar1=GAZE_DIR)
            vy = g2.t()
            nc.vector.tensor_scalar_mul(out=vy, in0=ax, scalar1=-GAZE_DIR)
            cdot = g2.t()
            nc.vector.tensor_scalar_mul(out=cdot, in0=az, scalar1=GAZE_DIR)
            fden = g2.t()
            nc.vector.tensor_scalar_add(out=fden, in0=cdot, scalar1=1.0 + 1e-8)
            f = g2.t()
            nc.vector.reciprocal(out=f, in_=fden)
            vv = g2.mac(vy, vy, g2.mul(vx, vx))
            fvv = g2.mul(f, vv)
            dd = g2.t()  # 1 - f*vv
            nc.vector.tensor_scalar(
                out=dd, in0=fvv, scalar1=-1.0, scalar2=1.0, op0=ALU.mult, op1=ALU.add
            )
            fxy = g2.mul(g2.mul(vx, vy), f)
            Rl = geop.tile([128, 9], F32)
            nc.vector.tensor_tensor(
                out=Rl[:, 0:1], in0=dd, in1=g2.mul(f, g2.mul(vx, vx)), op=ALU.add
            )
            nc.vector.tensor_tensor(
                out=Rl[:, 4:5], in0=dd, in1=g2.mul(f, g2.mul(vy, vy)), op=ALU.add
            )
            nc.vector.tensor_copy(out=Rl[:, 8:9], in_=dd)
            nc.vector.tensor_copy(out=Rl[:, 1:2], in_=fxy)
            nc.vector.tensor_copy(out=Rl[:, 3:4], in_=fxy)
            nc.vector.tensor_copy(out=Rl[:, 2:3], in_=vy)
            nc.vector.tensor_scalar_mul(out=Rl[:, 5:6], in0=vx, scalar1=-1.0)
            nc.vector.tensor_scalar_mul(out=Rl[:, 6:7], in0=vy, scalar1=-1.0)
            nc.vector.tensor_copy(out=Rl[:, 7:8], in_=vx)
            # eyeball rotation from latent rot2 (az=0), stacked l/r
            aa2 = geop.tile([128, 3], F32)
            nc.vector.memset(aa2, 0.0)
            nc.vector.tensor_copy(out=aa2[0:B, 0:2], in_=lat[:, 552:554])
            nc.sync.dma_start(out=aa2[B:128, 0:2], in_=lat[:, 554:556])
            R2 = axis_angle_R(nc, g2, aa2, "e_", halfpi)
            # gaze = GAZE_DIR * R2[2,:]
            gz = geop.tile([128, 3], F32)
            nc.vector.tensor_scalar_mul(out=gz, in0=R2[:, 6:9], scalar1=GAZE_DIR)
            # M = Rl @ R2
            M = geop.tile([128, 9], F32)
            for l in range(3):
                for i in range(3):
                    t = g2.mul(Rl[:, 3 * l:3 * l + 1], R2[:, i:i + 1])
                    t = g2.mac(R2[:, 3 + i:4 + i], Rl[:, 3 * l + 1:3 * l + 2], t)
                    t = g2.mac(R2[:, 6 + i:7 + i], Rl[:, 3 * l + 2:3 * l + 3], t)
                    nc.vector.tensor_copy(out=M[:, 3 * l + i:3 * l + i + 1], in_=t)
            # offe_i = c_i - sum_l c_l M[l,i]
            offe = geop.tile([128, 3], F32)
            for i in range(3):
                t = g2.mul(c3[:, 0:1], M[:, i:i + 1])
                t = g2.mac(c3[:, 1:2], M[:, 3 + i:4 + i], t)
                t = g2.mac(c3[:, 2:3], M[:, 6 + i:7 + i], t)
                nc.vector.tensor_tensor(
                    out=offe[:, i:i + 1], in0=c3[:, i:i + 1], in1=t, op=ALU.subtract
                )
            # apply to both eye slices
            es2 = geop.tile([128, 3, EW], F32)
            for i in range(3):
                nc.vector.tensor_scalar(
                    out=es2[:, i, :], in0=es[:, 0, :],
                    scalar1=M[:, i:i + 1], scalar2=offe[:, i:i + 1],
                    op0=ALU.mult, op1=ALU.add,
                )
                for l in (1, 2):
                    nc.vector.scalar_tensor_tensor(
                        out=es2[:, i, :], in0=es[:, l, :],
                        scalar=M[:, 3 * l + i:3 * l + i + 1], in1=es2[:, i, :],
                        op0=ALU.mult, op1=ALU.add,
                    )
            for i in range(3):
                nc.vector.tensor_copy(out=rt[:, i, l_lo:l_lo + EW], in_=es2[0:B, i, :])
            nc.sync.dma_start(out=rt[:, :, r_lo:r_lo + EW], in_=es2[B:128, :, :])
            # unpack right-eye centre/gaze down to rows 0:64 for the solve
            rc64 = geop.tile([B, 3], F32)
            nc.sync.dma_start(out=rc64, in_=c3[B:128, :])
            rg64 = geop.tile([B, 3], F32)
            nc.sync.dma_start(out=rg64, in_=gz[B:128, :])
            lc = c3[0:B, :]
            lg = gz[0:B, :]
            rc = rc64
            rg = rg64

            # face centre from landmarks
            fc = geop.tile([B, 3], F32)
            for i in range(3):
                t4 = g.add(rt[:, i, idx4[0]:idx4[0] + 1], rt[:, i, idx4[1]:idx4[1] + 1])
                t4 = g.add(t4, rt[:, i, idx4[2]:idx4[2] + 1])
                t4 = g.add(t4, rt[:, i, idx4[3]:idx4[3] + 1])
                t2 = g.add(rt[:, i, idx2[0]:idx2[0] + 1], rt[:, i, idx2[1]:idx2[1] + 1])
                # fc = t4/4/2 + t2/2/2
                o = g.t()
                nc.vector.tensor_scalar_mul(out=o, in0=t4, scalar1=0.125)
                nc.vector.scalar_tensor_tensor(
                    out=fc[:, i:i + 1], in0=t2, scalar=0.25, in1=o,
                    op0=ALU.mult, op1=ALU.add,
                )

            # gaze intersection (Cramer)
            d = [g.sub(rc[:, i:i + 1], lc[:, i:i + 1]) for i in range(3)]
            c0 = [lg[:, i:i + 1] for i in range(3)]
            c1 = []
            for i in range(3):
                o = g.t()
                nc.vector.tensor_scalar_mul(out=o, in0=rg[:, i:i + 1], scalar1=-1.0)
                c1.append(o)
            # c2 = rg x lg
            c2 = list(g.cross3(rg[:, 0:1], rg[:, 1:2], rg[:, 2:3],
                               lg[:, 0:1], lg[:, 1:2], lg[:, 2:3]))
            # w = c1 x c2 ; det = c0.w ; num0 = d.w
            w = g.cross3(*c1, *c2)
            det = g.dot3(*c0, *w)
            num0 = g.dot3(*d, *w)
            # w2 = d x c2 ; num1 = c0.w2  (det with col1 replaced by d)
            w2 = g.cross3(*d, *c2)
            num1 = g.dot3(*c0, *w2)
            rdet = g.t()
            nc.vector.reciprocal(out=rdet, in_=det)
            sol0 = g.mul(num0, rdet)
            sol1 = g.mul(num1, rdet)
            # gp_l = l_c + sol0*lg ; gp_r = r_c + sol1*rg ; gp_mid
            gpl = geop.tile([B, 3], F32)
            gpr = geop.tile([B, 3], F32)
            gpm = geop.tile([B, 3], F32)
            for i in range(3):
                nc.vector.scalar_tensor_tensor(
                    out=gpl[:, i:i + 1], in0=lg[:, i:i + 1], scalar=sol0,
                    in1=lc[:, i:i + 1], op0=ALU.mult, op1=ALU.add,
                )
                nc.vector.scalar_tensor_tensor(
                    out=gpr[:, i:i + 1], in0=rg[:, i:i + 1], scalar=sol1,
                    in1=rc[:, i:i + 1], op0=ALU.mult, op1=ALU.add,
                )
            nc.vector.tensor_tensor(out=gpm, in0=gpl, in1=gpr, op=ALU.add)
            nc.vector.tensor_scalar_mul(out=gpm, in0=gpm, scalar1=0.5)
            dff = geop.tile([B, 3], F32)
            nc.vector.tensor_tensor(out=dff, in0=gpl, in1=gpr, op=ALU.subtract)
            nc.vector.tensor_tensor(out=dff, in0=dff, in1=dff, op=ALU.mult)
            d2 = g.t()
            nc.vector.tensor_reduce(out=d2, in_=dff, axis=AX.X, op=ALU.add)
            dist = g.t()
            nc.scalar.activation(out=dist, in_=d2, func=ACTF.Sqrt)
            # far points l_c + 1000*lg
            farl = geop.tile([B, 3], F32)
            farr = geop.tile([B, 3], F32)
            for i in range(3):
                nc.vector.scalar_tensor_tensor(
                    out=farl[:, i:i + 1], in0=lg[:, i:i + 1], scalar=1000.0,
                    in1=lc[:, i:i + 1], op0=ALU.mult, op1=ALU.add,
                )
                nc.vector.scalar_tensor_tensor(
                    out=farr[:, i:i + 1], in0=rg[:, i:i + 1], scalar=1000.0,
                    in1=rc[:, i:i + 1], op0=ALU.mult, op1=ALU.add,
                )

            # projection of face verts
            cam = geop.tile([B, 12], F32)
            nc.sync.dma_start(out=cam, in_=cam_p[:, :])
            with tc.tile_pool(name="imgp", bufs=1) as imgp:
                img = imgp.tile([B, 3, VM], F32)
                for i in (2, 0, 1):  # z first (feeds the clamp chain on DVE)
                    eng = nc.vector
                    eng.tensor_scalar(
                        out=img[:, i, :], in0=rt[:, 0, 0:VM],
                        scalar1=cam[:, 4 * i:4 * i + 1], scalar2=cam[:, 4 * i + 3:4 * i + 4],
                        op0=ALU.mult, op1=ALU.add,
                    )
                    for l in (1, 2):
                        eng.scalar_tensor_tensor(
                            out=img[:, i, :], in0=rt[:, l, 0:VM],
                            scalar=cam[:, 4 * i + l:4 * i + l + 1], in1=img[:, i, :],
                            op0=ALU.mult, op1=ALU.add,
                        )
                with tc.tile_pool(name="ztmp", bufs=1) as ztp:
                    az_ = ztp.tile([B, VM], F32)
                    nc.scalar.activation(out=az_, in_=img[:, 2, :], func=ACTF.Abs)
                    nc.vector.tensor_scalar_max(out=az_, in0=az_, scalar1=1e-3)
                    sg = ztp.tile([B, VM], F32)
                    nc.vector.tensor_scalar(
                        out=sg, in0=img[:, 2, :], scalar1=0.0, scalar2=None, op0=ALU.is_ge
                    )
                    nc.vector.tensor_scalar(
                        out=sg, in0=sg, scalar1=2.0, scalar2=1.0,
                        op0=ALU.mult, op1=ALU.subtract,
                    )
                    nc.vector.tensor_tensor(out=sg, in0=sg, in1=az_, op=ALU.mult)
                    nc.vector.reciprocal(out=az_, in_=sg)
                    nc.vector.tensor_tensor(
                        out=img[:, 0, :], in0=img[:, 0, :], in1=az_, op=ALU.mult
                    )
                    nc.vector.tensor_tensor(
                        out=img[:, 1, :], in0=img[:, 1, :], in1=az_, op=ALU.mult
                    )

                # landmark gather + tail assembly
                fl = geop.tile([B, 3, 68], F32)
                def _cp(k, out, in_):
                    e = k % 3
                    if e == 0:
                        nc.vector.tensor_copy(out=out, in_=in_)
                    elif e == 1:
                        nc.scalar.copy(out=out, in_=in_)
                    else:
                        nc.gpsimd.tensor_copy(out=out, in_=in_)

                for j, idx in enumerate(fl_idx):
                    for i in range(3):
                        _cp(j * 3 + i, fl[:, i, j:j + 1], rt[:, i, idx:idx + 1])
                tail = geop.tile([B, 3, 11], F32)
                for i in range(3):
                    pieces = [
                        lc[:, i:i + 1], rc[:, i:i + 1], fc[:, i:i + 1],
                        gpl[:, i:i + 1], gpr[:, i:i + 1], gpm[:, i:i + 1],
                        farl[:, i:i + 1], farr[:, i:i + 1],
                        lg[:, i:i + 1], rg[:, i:i + 1], dist,
                    ]
                    for j, src in enumerate(pieces):
                        _cp(i * 11 + j, tail[:, i, j:j + 1], src)

                # output DMAs
                for i in range(3):
                    nc.sync.dma_start(out=out_p[:, i, 0:VM], in_=rt[:, i, 0:VM])
                    nc.sync.dma_start(out=out_p[:, i, VM:2 * VM], in_=img[:, i, :])
                    nc.sync.dma_start(
                        out=out_p[:, i, 2 * VM:2 * VM + 68], in_=fl[:, i, :]
                    )
                    nc.sync.dma_start(
                        out=out_p[:, i, 2 * VM + 68:NOUT], in_=tail[:, i, :]
                    )
    _legalize_waits(nc)
    return nc


def _prep(inputs):
    x = np.ascontiguousarray(inputs["x"].reshape(B, DIN), dtype=np.float32)
    enc_W = np.asarray(inputs["enc_W"], dtype=np.float32)
    basis_np = np.asarray(inputs["shape_basis"], dtype=np.float32)
    tmpl_np = np.asarray(inputs["v_template"], dtype=np.float32)
    enc_b = np.concatenate([
        np.asarray(inputs["enc_b"], dtype=np.float32).reshape(1, LAT),
        np.full((1, B), 1.0 / NCORES, np.float32),
        np.ones((1, B), np.float32),
        tmpl_np.mean(axis=0).reshape(1, 3),
    ], axis=1)
    bmean_full = basis_np.mean(axis=1)  # [400, 3]
    bmean = np.zeros((128, 12), np.float32)
    for ki, (k0, kw) in enumerate([(0, 128), (128, 128), (256, 128), (384, 16)]):
        bmean[:kw, ki * 3:ki * 3 + 3] = bmean_full[k0:k0 + kw]
    tmpl = np.ascontiguousarray(
        np.asarray(inputs["v_template"], dtype=np.float32).T
    )  # [3, V]
    basis = np.ascontiguousarray(
        np.asarray(inputs["shape_basis"], dtype=np.float32).transpose(0, 2, 1)
    )  # [400, 3, V]
    cam = np.ascontiguousarray(
        np.asarray(inputs["camera_parameters"], dtype=np.float32).reshape(B, 12)
    )
    lm = np.asarray(inputs["landmarks"])
    mlm = np.asarray(inputs["masked_landmarks"])
    fmask = np.asarray(inputs["face_mask"])
    lmask = np.asarray(inputs["left_eyeball_mask"])
    rmask = np.asarray(inputs["right_eyeball_mask"])
    assert np.array_equal(lmask, np.arange(lmask[0], lmask[0] + 546)), "lmask not contiguous"
    assert np.array_equal(rmask, np.arange(rmask[0], rmask[0] + 546)), "rmask not contiguous"
    fl_idx = [int(fmask[i]) for i in mlm]
    idx4 = [int(lm[j]) for j in (19, 22, 25, 28)]
    idx2 = [int(lm[j]) for j in (14, 18)]
    return (x, enc_W, enc_b, bmean, tmpl, basis, cam, fl_idx, idx4, idx2,
            int(lmask[0]), int(rmask[0]))


def _run(inputs, trace=False):
    (x, enc_W, enc_b, bmean, tmpl, basis, cam, fl_idx, idx4, idx2, l_lo, r_lo) = _prep(inputs)
    nc = build_graph(fl_idx, idx4, idx2, l_lo, r_lo)
    in_maps = []
    for c in range(NCORES):
        k0 = c * KSH
        in_maps.append({
            "x_sh": np.ascontiguousarray(x[:, k0:k0 + KSH].T),
            "w_sh": np.ascontiguousarray(enc_W[k0:k0 + KSH, :]),
            "enc_b": enc_b,
            "bmean": bmean,
            "tmpl": tmpl,
            "basis": basis,
            "cam": cam,
        })
    res = run_bass_kernel_spmd(
        nc, in_maps, core_ids=list(range(NCORES)), trace=trace
    )
    out = res.results[0]["out"]  # [B, 3, NOUT]
    return np.ascontiguousarray(out.transpose(0, 2, 1)), res


def kernel(**inputs):
    out, _ = _run(inputs, trace=False)
    return out



# revision 2
# speedup vs baseline: 1.7914x; 1.7914x over previous
"""Trainium2 Bass kernel for nn_Autoencoder_65223373357102 (FLAME-style autoencoder).

Strategy (v2):
  Phase 1 (8-way tensor parallel): encoder GEMM sharded along K. W is packed on
  the host to the 411 *used* latent columns (0:400 shape + 545:556 pose), f32
  (the projection divides by z clamped at 1e-3, so vmk needs ~1e-5 relative
  accuracy -> no bf16/f32r anywhere on the shape path). x is SBUF-resident; W
  streams in 21 pre-tiled contiguous chunks on two DMA queues. Bias (scaled
  1/8) and a constant 1/8 lane (col 411) are folded into the PSUM accumulation;
  the AllReduce of [64,412] then yields latent + an exact 1.0 in col 411 that
  phase 2 uses as the template coefficient.
  Phase 2 (8-way vertex parallel): each core computes only its 448 of the 3500
  face verts plus 72 synthetic columns (68 landmarks, l/r eye means, face
  centre, vmean) via a [64,400+]@[400+,1560] GEMM from host-gathered basis
  columns. Everything the reference does to the eye vertex slices is dead code
  w.r.t. the output (only the eye means and gaze rotations survive), so it is
  skipped. shape_p is transposed on the PE (identity matmul). Per-core output
  [64,3,975] is stitched to the full [64,7079,3] on the host.
"""
import sys
import types

sys.path.insert(0, "/opt/trn_rl_repo")

import numpy as np


def _ensure_ntff_hook():
    """Provide antenv.axon_hooks + install the ctypes NTFF profile hook so
    run_bass_kernel_spmd(trace=True) can pull a neuron-profile under axon."""
    name = "antenv.axon_hooks"
    if name not in sys.modules:
        mod = types.ModuleType(name)
        mod._HOOK = None

        def set_axon_ntff_profile_hook(hook):
            mod._HOOK = hook

        def get_axon_ntff_profile_hook():
            return mod._HOOK

        mod.set_axon_ntff_profile_hook = set_axon_ntff_profile_hook
        mod.get_axon_ntff_profile_hook = get_axon_ntff_profile_hook
        sys.modules[name] = mod
        try:
            import antenv

            antenv.axon_hooks = mod
        except ImportError:
            pass
    mod = sys.modules[name]
    if mod.get_axon_ntff_profile_hook() is None:
        try:
            from trn_agent_boot.trn_boot import _ntff_profile_via_ctypes

            hook = _ntff_profile_via_ctypes("/opt/axon/libaxon_pjrt.so")
            if hook is not None:
                mod.set_axon_ntff_profile_hook(hook)
        except Exception:
            pass


_ensure_ntff_hook()

from concourse import bass, mybir, tile
from concourse.bass_utils import run_bass_kernel_spmd

F32 = mybir.dt.float32
ALU = mybir.AluOpType
ACTF = mybir.ActivationFunctionType
AX = mybir.AxisListType

B = 64
V = 5023
VM = 3500
LAT = 556
DIN = 3 * 224 * 224  # 150528
NCORES = 8
KSH = DIN // NCORES  # 18816
KT = KSH // 128  # 147 k-tiles
TPC = 7  # k-tiles per W chunk
NCH = KT // TPC  # 21 chunks
NCOLS = 411  # packed latent cols: 0:400 + 545:556
NOUT = 2 * VM + 68 + 11  # 7079
SL = 448  # verts per core (last core: 364 real + pad)
PL = SL + 68 + 4  # per-plane block: slice, fl, lme, rme, fc, vmean = 520
N2 = 3 * PL  # 1560
GAZE_DIR = -1.0
HALF_PI = 1.5707963267948966
# packed pose col offsets (orig 545:556 -> packed 400:411)
P_ROT, P_T, P_SC, P_LR, P_RR = 400, 403, 406, 407, 409


class Geo:
    """Helper for tiny per-batch scalar ops on [rows,1] tiles."""

    _uid = [0]

    def __init__(self, nc, pool, rows=B):
        self.nc = nc
        self.pool = pool
        self.rows = rows

    def t(self, cols=1):
        Geo._uid[0] += 1
        return self.pool.tile([self.rows, cols], F32, name=f"g{Geo._uid[0]}_{cols}")

    def mul(self, a, b):
        o = self.t()
        self.nc.vector.tensor_tensor(out=o, in0=a, in1=b, op=ALU.mult)
        return o

    def add(self, a, b):
        o = self.t()
        self.nc.vector.tensor_tensor(out=o, in0=a, in1=b, op=ALU.add)
        return o

    def sub(self, a, b):
        o = self.t()
        self.nc.vector.tensor_tensor(out=o, in0=a, in1=b, op=ALU.subtract)
        return o

    def mac(self, a, s, acc):
        """(a * s) + acc, s is a [rows,1] AP scalar."""
        o = self.t()
        self.nc.vector.scalar_tensor_tensor(
            out=o, in0=a, scalar=s, in1=acc, op0=ALU.mult, op1=ALU.add
        )
        return o

    def dot3(self, ax, ay, az, bx, by, bz):
        o = self.mul(ax, bx)
        o = self.mac(ay, by, o)
        o = self.mac(az, bz, o)
        return o

    def cross3(self, ax, ay, az, bx, by, bz):
        cx = self.sub(self.mul(ay, bz), self.mul(az, by))
        cy = self.sub(self.mul(az, bx), self.mul(ax, bz))
        cz = self.sub(self.mul(ax, by), self.mul(ay, bx))
        return cx, cy, cz


def axis_angle_R(nc, g, aa3, pfx, halfpi):
    R_ = g.rows
    """aa3: [rows,3] axis-angle tile -> R [rows,9] tile, R[l,i] at col l*3+i."""
    pool = g.pool
    sq = pool.tile([R_, 3], F32, name=pfx + "aaR_sq")
    nc.vector.tensor_tensor(out=sq, in0=aa3, in1=aa3, op=ALU.mult)
    th2 = g.t()
    nc.vector.tensor_reduce(out=th2, in_=sq, axis=AX.X, op=ALU.add)
    theta = g.t()
    nc.scalar.activation(out=theta, in_=th2, func=ACTF.Sqrt)
    thm = g.t()
    nc.vector.tensor_scalar_max(out=thm, in0=theta, scalar1=1e-8)
    rth = g.t()
    nc.vector.reciprocal(out=rth, in_=thm)
    axis3 = pool.tile([R_, 3], F32, name=pfx + "aaR_axis")
    nc.vector.tensor_scalar_mul(out=axis3, in0=aa3, scalar1=rth)
    s = g.t()
    nc.scalar.activation(out=s, in_=theta, func=ACTF.Sin)
    c = g.t()
    nc.scalar.activation(out=c, in_=theta, func=ACTF.Sin, bias=halfpi)
    omc = g.t()
    nc.vector.tensor_scalar(
        out=omc, in0=c, scalar1=-1.0, scalar2=1.0, op0=ALU.mult, op1=ALU.add
    )
    ax, ay, az = axis3[:, 0:1], axis3[:, 1:2], axis3[:, 2:3]
    asq = pool.tile([R_, 3], F32, name=pfx + "aaR_asq")
    nc.vector.tensor_tensor(out=asq, in0=axis3, in1=axis3, op=ALU.mult)
    R = pool.tile([R_, 9], F32, name=pfx + "aaR_R")
    dmul = pool.tile([R_, 3], F32, name=pfx + "aaR_dmul")
    nc.vector.tensor_scalar_mul(out=dmul, in0=asq, scalar1=omc)
    sa = pool.tile([R_, 3], F32, name=pfx + "aaR_sa")
    nc.vector.tensor_scalar_mul(out=sa, in0=axis3, scalar1=s)
    sax, say, saz = sa[:, 0:1], sa[:, 1:2], sa[:, 2:3]
    mxy = g.mul(g.mul(ax, ay), omc)
    mxz = g.mul(g.mul(ax, az), omc)
    myz = g.mul(g.mul(ay, az), omc)
    for l in range(3):
        nc.vector.tensor_tensor(
            out=R[:, 4 * l:4 * l + 1], in0=dmul[:, l:l + 1], in1=c, op=ALU.add
        )
    nc.vector.tensor_tensor(out=R[:, 1:2], in0=mxy, in1=saz, op=ALU.subtract)  # R01
    nc.vector.tensor_tensor(out=R[:, 2:3], in0=mxz, in1=say, op=ALU.add)  # R02
    nc.vector.tensor_tensor(out=R[:, 3:4], in0=mxy, in1=saz, op=ALU.add)  # R10
    nc.vector.tensor_tensor(out=R[:, 5:6], in0=myz, in1=sax, op=ALU.subtract)  # R12
    nc.vector.tensor_tensor(out=R[:, 6:7], in0=mxz, in1=say, op=ALU.subtract)  # R20
    nc.vector.tensor_tensor(out=R[:, 7:8], in0=myz, in1=sax, op=ALU.add)  # R21
    return R


_ENG_ATTR = {
    "SP": "sync", "Pool": "gpsimd", "PE": "tensor",
    "DVE": "vector", "Activation": "scalar",
}


def _legalize_waits(nc):
    """This walrus accepts only one sync-wait slot per instruction; move extra
    waits onto same-engine NoOps inserted right before the instruction."""
    import concourse.mybir as _mybir

    def make_nop(engine):
        eng = getattr(nc, _ENG_ATTR[engine.name])
        bi = eng.nop(nofuse=True)
        mi = bi.ins
        for bb in nc.main_func.blocks:
            if bb.instructions and bb.instructions[-1].name == mi.name:
                bb.instructions.pop()
                break
        mi.engine = engine
        return mi

    for bb in nc.main_func.blocks:
        snapshot = list(bb.instructions)
        newlist = []
        changed = False
        for inst in snapshot:
            si = inst.sync_info
            waits = list(si.on_wait) if (si and si.on_wait) else []
            if (
                len(waits) > 1
                and not inst.name.startswith("barrier")
                and inst.engine is not None
                and getattr(inst.engine, "name", None) in _ENG_ATTR
            ):
                for w in waits[:-1]:
                    nop = make_nop(inst.engine)
                    nop.sync_info = _mybir.SyncInfo(on_wait=[w], on_update=[])
                    newlist.append(nop)
                inst.sync_info = _mybir.SyncInfo(
                    on_wait=[waits[-1]], on_update=list(si.on_update)
                )
                changed = True
            newlist.append(inst)
        if changed:
            bb.instructions[:] = newlist
    return nc


XPARTS = [25, 25, 25, 25, 25, 22]  # k-tile split of the resident x shard


def build_graph():
    nc = bass.Bass(target_bir_lowering=False)

    x_p = nc.declare_dram_parameter("xw", [128, KT, B], F32, isOutput=False)
    w_p = nc.declare_dram_parameter("wch", [NCH, 128, TPC, NCOLS], F32, isOutput=False)
    b_p = nc.declare_dram_parameter("bvec", [1, NCOLS + 1], F32, isOutput=False)
    bas_p = nc.declare_dram_parameter("basis", [128, 4, N2], F32, isOutput=False)
    cam_p = nc.declare_dram_parameter("cam", [B, 12], F32, isOutput=False)
    eye_p = nc.declare_dram_parameter("eye", [B, B], F32, isOutput=False)
    out_p = nc.declare_dram_parameter("out", [B, 3, 2 * SL + 71 + 8], F32, isOutput=True)

    ar_in = nc.dram_tensor("ar_in", [B, NCOLS + 1], F32)
    ar_out = nc.dram_tensor("ar_out", [B, NCOLS + 1], F32, addr_space="Shared")

    with tile.TileContext(nc) as tc:
        with (
            tc.tile_pool(name="consts", bufs=1) as consts,
            tc.tile_pool(name="xres", bufs=1) as xres,
            tc.tile_pool(name="latp", bufs=1) as latp,
            tc.tile_pool(name="geop", bufs=1) as geop,
            tc.tile_pool(name="planep", bufs=1) as planep,
            tc.tile_pool(name="dum", bufs=1, space="PSUM") as dum,
        ):
            # ---- const / prefetch loads ----
            eye_sb = consts.tile([B, B], F32)
            nc.scalar.dma_start(out=eye_sb, in_=eye_p[:, :])
            cam = consts.tile([B, 12], F32)
            nc.scalar.dma_start(out=cam, in_=cam_p[:, :])
            b_sb = consts.tile([1, NCOLS + 1], F32)
            nc.scalar.dma_start(out=b_sb, in_=b_p[:, :])
            ones1 = consts.tile([1, B], F32)
            nc.vector.memset(ones1, 1.0)
            halfpi = consts.tile([128, 1], F32)
            nc.vector.memset(halfpi, HALF_PI)
            lat = latp.tile([B, 416], F32)
            nc.vector.memset(lat, 0.0)

            # resident x shard, split into parts so PE can start after part 0
            xts = []
            off = 0
            for pi, n in enumerate(XPARTS):
                xt = xres.tile([128, n, B], F32, name=f"xt{pi}")
                nc.gpsimd.dma_start(out=xt, in_=x_p[:, off:off + n, :])
                xts.append((off, n, xt))
                off += n
            # phase-2 basis block (prefetch; lands during phase 1)
            basis_sb = planep.tile([128, 4, N2], F32)
            nc.gpsimd.dma_start(out=basis_sb, in_=bas_p[:, :, :])

            def xap(k):
                for off, n, xt in xts:
                    if k < off + n:
                        return xt[:, k - off, :]
                raise IndexError(k)

            d1 = dum.tile([1, 1], F32)

            # ---------------- Phase 1: encoder GEMM ----------------
            with (
                tc.tile_pool(name="wts", bufs=3) as wts,
                tc.tile_pool(name="encp", bufs=1, space="PSUM") as encp,
            ):
                pe = encp.tile([B, NCOLS + 1], F32)
                nc.tensor.matmul(
                    d1, lhsT=xts[0][2][:, 0, 0:1], rhs=xts[0][2][:, 0, 0:1],
                    start=True, stop=True, skip_group_check=True,
                )
                for ci in range(NCH):
                    w_c = wts.tile([128, TPC, NCOLS], F32)
                    eng = nc.sync if ci % 2 == 0 else nc.scalar
                    eng.dma_start(out=w_c, in_=w_p[ci])
                    for t in range(TPC):
                        k = ci * TPC + t
                        nc.tensor.matmul(
                            pe[:, 0:NCOLS],
                            lhsT=xap(k),
                            rhs=w_c[:, t, :],
                            start=(k == 0),
                            stop=False,
                        )
                # bias (scaled 1/8) + constant 1/8 lane in col 411
                nc.tensor.matmul(
                    d1, lhsT=b_sb[0:1, 0:1], rhs=b_sb[0:1, 0:1],
                    start=True, stop=True, skip_group_check=True,
                )
                nc.tensor.matmul(
                    pe, lhsT=ones1, rhs=b_sb, start=False, stop=True,
                )
                lat1 = latp.tile([B, NCOLS + 1], F32)
                nc.vector.tensor_copy(out=lat1, in_=pe)
                nc.sync.dma_start(out=ar_in[:, :], in_=lat1)

            nc.gpsimd.collective_compute(
                "AllReduce",
                ALU.add,
                replica_groups=[list(range(NCORES))],
                ins=[ar_in.ap().opt()],
                outs=[ar_out.ap().opt()],
            )
            nc.sync.dma_start(out=lat[:, 0:NCOLS + 1], in_=ar_out[:, :])

            # ---------------- Phase 1.5: transpose shape params on PE ----------
            with tc.tile_pool(name="trps", bufs=1, space="PSUM") as trps:
                trp = trps.tile([128, 4, B], F32)
                nc.tensor.matmul(
                    d1, lhsT=eye_sb[0:1, 0:1], rhs=eye_sb[0:1, 0:1],
                    start=True, stop=True, skip_group_check=True,
                )
                for kt in range(3):
                    nc.tensor.matmul(
                        trp[:, kt, :], lhsT=lat[:, kt * 128:(kt + 1) * 128],
                        rhs=eye_sb, is_transpose=True,
                        start=True, stop=True, skip_group_check=True,
                    )
                nc.tensor.matmul(
                    trp[0:32, 3, :], lhsT=lat[:, 384:416],
                    rhs=eye_sb, is_transpose=True,
                    start=True, stop=True, skip_group_check=True,
                )
                spT = latp.tile([128, 4, B], F32)
                nc.scalar.copy(out=spT, in_=trp)

            # ---------------- Phase 2: blendshape GEMM (V-sharded) -------------
            g = Geo(nc, geop)
            vpre = planep.tile([B, N2], F32)
            NSPL2 = [(0, 512), (512, 512), (1024, 512), (1536, N2 - 1536)]
            with tc.tile_pool(name="p2ps", bufs=1, space="PSUM") as p2ps:
                pvs = [
                    p2ps.tile([B, n], F32, name=f"pv{j}", tag=f"pv{j}")
                    for j, (_, n) in enumerate(NSPL2)
                ]
                nc.tensor.matmul(
                    d1, lhsT=basis_sb[0:1, 0, 0:1], rhs=basis_sb[0:1, 0, 0:1],
                    start=True, stop=True, skip_group_check=True,
                )
                for j, (n0, n) in enumerate(NSPL2):
                    for kt in range(4):
                        rows = 128 if kt < 3 else 32
                        nc.tensor.matmul(
                            pvs[j],
                            lhsT=spT[0:rows, kt, :],
                            rhs=basis_sb[0:rows, kt, n0:n0 + n],
                            start=(kt == 0),
                            stop=(kt == 3),
                        )
                for j, (n0, n) in enumerate(NSPL2):
                    nc.scalar.copy(out=vpre[:, n0:n0 + n], in_=pvs[j])

            # face rotation matrix, scaled
            Rf = axis_angle_R(nc, g, lat[:, P_ROT:P_ROT + 3], "f_", halfpi[:B, :])
            fs = g.t()
            nc.vector.tensor_scalar_add(out=fs, in0=lat[:, P_SC:P_SC + 1], scalar1=1.0)
            Rs = geop.tile([B, 9], F32)
            nc.vector.tensor_scalar_mul(out=Rs, in0=Rf, scalar1=fs)
            # offsets: off_i = face_t_i - sum_l vms_l*Rs[l,i]
            off3 = geop.tile([B, 3], F32)
            for i in range(3):
                t = g.mul(vpre[:, 519:520], Rs[:, i:i + 1])
                t = g.mac(vpre[:, 520 + 519:520 + 520], Rs[:, 3 + i:4 + i], t)
                t = g.mac(vpre[:, 1040 + 519:1040 + 520], Rs[:, 6 + i:7 + i], t)
                nc.vector.tensor_tensor(
                    out=off3[:, i:i + 1], in0=lat[:, P_T + i:P_T + i + 1], in1=t,
                    op=ALU.subtract,
                )

            # eyeball rotations (l rows 0:64, r rows 64:128)
            g2 = Geo(nc, geop, rows=128)
            aa2 = geop.tile([128, 3], F32)
            nc.vector.memset(aa2, 0.0)
            nc.vector.tensor_copy(out=aa2[0:B, 0:2], in_=lat[:, P_LR:P_LR + 2])
            nc.sync.dma_start(out=aa2[B:128, 0:2], in_=lat[:, P_RR:P_RR + 2])
            R2 = axis_angle_R(nc, g2, aa2, "e_", halfpi)
            gz = geop.tile([128, 3], F32)
            nc.vector.tensor_scalar_mul(out=gz, in0=R2[:, 6:9], scalar1=GAZE_DIR)
            rg64 = geop.tile([B, 3], F32)
            nc.sync.dma_start(out=rg64, in_=gz[B:128, :])

            # rotate + translate all plane blocks
            rt = planep.tile([B, 3, PL], F32)
            for i in range(3):
                nc.vector.tensor_scalar(
                    out=rt[:, i, :], in0=vpre[:, 0:PL],
                    scalar1=Rs[:, i:i + 1], scalar2=off3[:, i:i + 1],
                    op0=ALU.mult, op1=ALU.add,
                )
                for l in (1, 2):
                    nc.vector.scalar_tensor_tensor(
                        out=rt[:, i, :], in0=vpre[:, l * PL:(l + 1) * PL],
                        scalar=Rs[:, 3 * l + i:3 * l + i + 1],
                        in1=rt[:, i, :],
                        op0=ALU.mult, op1=ALU.add,
                    )

            lc = [rt[:, i, SL + 68:SL + 69] for i in range(3)]
            rc = [rt[:, i, SL + 69:SL + 70] for i in range(3)]
            lg = [gz[0:B, i:i + 1] for i in range(3)]
            rg = [rg64[:, i:i + 1] for i in range(3)]

            # gaze intersection (Cramer)
            d = [g.sub(rc[i], lc[i]) for i in range(3)]
            c1 = []
            for i in range(3):
                o = g.t()
                nc.vector.tensor_scalar_mul(out=o, in0=rg[i], scalar1=-1.0)
                c1.append(o)
            c2 = list(g.cross3(*rg, *lg))
            w = g.cross3(*c1, *c2)
            det = g.dot3(*lg, *w)
            num0 = g.dot3(*d, *w)
            w2 = g.cross3(*d, *c2)
            num1 = g.dot3(*lg, *w2)
            rdet = g.t()
            nc.vector.reciprocal(out=rdet, in_=det)
            sol0 = g.mul(num0, rdet)
            sol1 = g.mul(num1, rdet)

            # tail block ge[:, i, j]: gp_l gp_r gp_mid far_l far_r lg rg dist
            ge = geop.tile([B, 3, 8], F32)
            gpl = geop.tile([B, 3], F32)
            gpr = geop.tile([B, 3], F32)
            for i in range(3):
                nc.vector.scalar_tensor_tensor(
                    out=gpl[:, i:i + 1], in0=lg[i], scalar=sol0,
                    in1=lc[i], op0=ALU.mult, op1=ALU.add,
                )
                nc.vector.scalar_tensor_tensor(
                    out=gpr[:, i:i + 1], in0=rg[i], scalar=sol1,
                    in1=rc[i], op0=ALU.mult, op1=ALU.add,
                )
                nc.vector.tensor_copy(out=ge[:, i, 0:1], in_=gpl[:, i:i + 1])
                nc.vector.tensor_copy(out=ge[:, i, 1:2], in_=gpr[:, i:i + 1])
                o = g.add(gpl[:, i:i + 1], gpr[:, i:i + 1])
                nc.vector.tensor_scalar_mul(out=ge[:, i, 2:3], in0=o, scalar1=0.5)
                nc.vector.scalar_tensor_tensor(
                    out=ge[:, i, 3:4], in0=lg[i], scalar=1000.0,
                    in1=lc[i], op0=ALU.mult, op1=ALU.add,
                )
                nc.vector.scalar_tensor_tensor(
                    out=ge[:, i, 4:5], in0=rg[i], scalar=1000.0,
                    in1=rc[i], op0=ALU.mult, op1=ALU.add,
                )
                nc.vector.tensor_copy(out=ge[:, i, 5:6], in_=lg[i])
                nc.vector.tensor_copy(out=ge[:, i, 6:7], in_=rg[i])
            dff = geop.tile([B, 3], F32)
            nc.vector.tensor_tensor(out=dff, in0=gpl, in1=gpr, op=ALU.subtract)
            nc.vector.tensor_tensor(out=dff, in0=dff, in1=dff, op=ALU.mult)
            d2 = g.t()
            nc.vector.tensor_reduce(out=d2, in_=dff, axis=AX.X, op=ALU.add)
            dist = g.t()
            nc.scalar.activation(out=dist, in_=d2, func=ACTF.Sqrt)
            for i in range(3):
                nc.vector.tensor_copy(out=ge[:, i, 7:8], in_=dist)

            # projection of this core's vert slice
            with tc.tile_pool(name="imgp", bufs=1) as imgp:
                img = imgp.tile([B, 3, SL], F32)
                for i in (2, 0, 1):  # z first (feeds the clamp chain)
                    nc.vector.tensor_scalar(
                        out=img[:, i, :], in0=rt[:, 0, 0:SL],
                        scalar1=cam[:, 4 * i:4 * i + 1],
                        scalar2=cam[:, 4 * i + 3:4 * i + 4],
                        op0=ALU.mult, op1=ALU.add,
                    )
                    for l in (1, 2):
                        nc.vector.scalar_tensor_tensor(
                            out=img[:, i, :], in0=rt[:, l, 0:SL],
                            scalar=cam[:, 4 * i + l:4 * i + l + 1], in1=img[:, i, :],
                            op0=ALU.mult, op1=ALU.add,
                        )
                az_ = imgp.tile([B, SL], F32)
                nc.scalar.activation(out=az_, in_=img[:, 2, :], func=ACTF.Abs)
                nc.vector.tensor_scalar_max(out=az_, in0=az_, scalar1=1e-3)
                sg = imgp.tile([B, SL], F32)
                nc.vector.tensor_scalar(
                    out=sg, in0=img[:, 2, :], scalar1=0.0, scalar2=None, op0=ALU.is_ge
                )
                nc.vector.tensor_scalar(
                    out=sg, in0=sg, scalar1=2.0, scalar2=1.0,
                    op0=ALU.mult, op1=ALU.subtract,
                )
                nc.vector.tensor_tensor(out=sg, in0=sg, in1=az_, op=ALU.mult)
                nc.vector.reciprocal(out=az_, in_=sg)
                nc.vector.tensor_tensor(
                    out=img[:, 0, :], in0=img[:, 0, :], in1=az_, op=ALU.mult
                )
                nc.vector.tensor_tensor(
                    out=img[:, 1, :], in0=img[:, 1, :], in1=az_, op=ALU.mult
                )

                # output DMAs
                nc.sync.dma_start(out=out_p[:, :, 0:SL], in_=rt[:, :, 0:SL])
                nc.scalar.dma_start(out=out_p[:, :, SL:2 * SL], in_=img)
                nc.sync.dma_start(
                    out=out_p[:, :, 2 * SL:2 * SL + 71], in_=rt[:, :, SL:SL + 71]
                )
                nc.scalar.dma_start(out=out_p[:, :, 2 * SL + 71:2 * SL + 79], in_=ge)
    _legalize_waits(nc)
    return nc


def _prep(inputs):
    f32 = np.float32
    x = np.ascontiguousarray(inputs["x"].reshape(B, DIN), dtype=f32)
    W = np.asarray(inputs["enc_W"], dtype=f32)
    Wp = np.concatenate([W[:, :400], W[:, 545:556]], axis=1)  # [DIN, 411]
    enc_b = np.asarray(inputs["enc_b"], dtype=f32)
    bp = np.concatenate([enc_b[:400], enc_b[545:556]])
    bvec = np.concatenate([bp / NCORES, np.array([1.0 / NCORES], f32)]).reshape(1, NCOLS + 1).astype(f32)
    tmpl = np.asarray(inputs["v_template"], dtype=f32)  # [V, 3]
    basis = np.asarray(inputs["shape_basis"], dtype=f32)  # [400, V, 3]
    cam = np.ascontiguousarray(
        np.asarray(inputs["camera_parameters"], dtype=f32).reshape(B, 12)
    )
    lm = np.asarray(inputs["landmarks"])
    mlm = np.asarray(inputs["masked_landmarks"])
    fmask = np.asarray(inputs["face_mask"])
    lmask = np.asarray(inputs["left_eyeball_mask"])
    rmask = np.asarray(inputs["right_eyeball_mask"])
    fl_idx = fmask[mlm]  # verts behind the 68 output landmarks
    idx4 = lm[np.array([19, 22, 25, 28])]
    idx2 = lm[np.array([14, 18])]

    # synthetic extra columns [400, 72, 3] / [72, 3]
    ex_b = np.concatenate([
        basis[:, fl_idx, :],
        basis[:, lmask, :].mean(axis=1, keepdims=True),
        basis[:, rmask, :].mean(axis=1, keepdims=True),
        (basis[:, idx4, :].mean(axis=1, keepdims=True)
         + basis[:, idx2, :].mean(axis=1, keepdims=True)) / 2.0,
        basis.mean(axis=1, keepdims=True),
    ], axis=1)
    ex_t = np.concatenate([
        tmpl[fl_idx],
        tmpl[lmask].mean(axis=0, keepdims=True),
        tmpl[rmask].mean(axis=0, keepdims=True),
        (tmpl[idx4].mean(axis=0, keepdims=True)
         + tmpl[idx2].mean(axis=0, keepdims=True)) / 2.0,
        tmpl.mean(axis=0, keepdims=True),
    ], axis=0)

    eye = np.eye(B, dtype=f32)
    in_maps = []
    for c in range(NCORES):
        k0 = c * KSH
        xs = x[:, k0:k0 + KSH].T  # [KSH, B]
        xw = np.ascontiguousarray(
            xs.reshape(KT, 128, B).transpose(1, 0, 2)
        )  # [128, KT, B]
        ws = Wp[k0:k0 + KSH]  # [KSH, 411]
        wch = np.ascontiguousarray(
            ws.reshape(NCH, TPC, 128, NCOLS).transpose(0, 2, 1, 3)
        )  # [NCH, 128, TPC, 411]

        lo = c * SL
        verts = fmask[lo:min(lo + SL, VM)]
        nsl = len(verts)
        blk = np.zeros((400, N2), f32)
        trow = np.zeros(N2, f32)
        for l in range(3):
            blk[:, l * PL:l * PL + nsl] = basis[:, verts, l]
            blk[:, l * PL + SL:l * PL + SL + 72] = ex_b[:, :, l]
            trow[l * PL:l * PL + nsl] = tmpl[verts, l]
            trow[l * PL + SL:l * PL + SL + 72] = ex_t[:, l]
        bh = np.zeros((128, 4, N2), f32)
        for kt in range(3):
            bh[:, kt, :] = blk[kt * 128:(kt + 1) * 128]
        bh[0:16, 3, :] = blk[384:400]
        bh[27, 3, :] = trow  # coefficient = exact 1.0 from AR col 411
        in_maps.append({
            "xw": xw,
            "wch": wch,
            "bvec": bvec,
            "basis": np.ascontiguousarray(bh),
            "cam": cam,
            "eye": eye,
        })
    return in_maps


def _run(inputs, trace=False):
    in_maps = _prep(inputs)
    nc = build_graph()
    res = run_bass_kernel_spmd(
        nc, in_maps, core_ids=list(range(NCORES)), trace=trace
    )
    full = np.empty((B, 3, NOUT), np.float32)
    for c in range(NCORES):
        r = res.results[c]["out"]  # [B, 3, 975]
        lo = c * SL
        w = min(SL, VM - lo)
        full[:, :, lo:lo + w] = r[:, :, 0:w]
        full[:, :, VM + lo:VM + lo + w] = r[:, :, SL:SL + w]
    r0 = res.results[0]["out"]
    full[:, :, 2 * VM:NOUT] = r0[:, :, 2 * SL:2 * SL + 79]
    return np.ascontiguousarray(full.transpose(0, 2, 1)), res


def kernel(**inputs):
    out, _ = _run(inputs, trace=False)
    return out


# revision 11
# speedup vs baseline: 2.0083x; 1.1211x over previous
"""Trainium2 Bass kernel for nn_Autoencoder_65223373357102 (FLAME-style autoencoder).

Strategy (v3):
  Phase 1 (8-way tensor parallel): encoder GEMM sharded along K, W packed to
  the 411 *used* latent columns. The fp32 GEMM is decomposed into three bf16
  passes (x_hi*W_hi + x_hi*W_lo + x_lo*W_hi, fp32 PSUM accumulation): bf16
  products are exact in fp32, so the latent error is ~4e-6 relative - inside
  the ~1e-5 budget set by the z-clamped projection - while the PE runs 1
  cycle/row instead of fp32's 4. x (hi+lo) is SBUF-resident; W streams in 21
  pre-tiled contiguous chunks on two DMA queues, deep-buffered so the NRT
  start barrier overlaps prefetch. Bias (scaled 1/8) and a constant 1/8 lane
  (col 411) are folded into the PSUM accumulation; the AllReduce of [64,412]
  then yields latent + an exact 1.0 in col 411 that phase 2 uses as the
  template coefficient.
  Phase 2 (8-way vertex parallel): each core computes only its 448 of the 3500
  face verts plus 72 synthetic columns (68 landmarks, l/r eye means, face
  centre, vmean) via an fp32 [64,400+]@[400+,1560] GEMM from host-gathered
  basis columns. Everything the reference does to the eye vertex slices is
  dead code w.r.t. the output (only the eye means and gaze rotations survive).
  shape_p is transposed on the PE (identity matmul). The eyeball-rotation
  chain runs on GpSimd in parallel with the DVE rotate/project chain. Per-core
  output [64,3,975] is stitched to the full [64,7079,3] on the host.
"""
import sys
import types

sys.path.insert(0, "/opt/trn_rl_repo")

import numpy as np
import ml_dtypes

BF = ml_dtypes.bfloat16


def _ensure_ntff_hook():
    """Provide antenv.axon_hooks + install the ctypes NTFF profile hook so
    run_bass_kernel_spmd(trace=True) can pull a neuron-profile under axon."""
    name = "antenv.axon_hooks"
    if name not in sys.modules:
        mod = types.ModuleType(name)
        mod._HOOK = None

        def set_axon_ntff_profile_hook(hook):
            mod._HOOK = hook

        def get_axon_ntff_profile_hook():
            return mod._HOOK

        mod.set_axon_ntff_profile_hook = set_axon_ntff_profile_hook
        mod.get_axon_ntff_profile_hook = get_axon_ntff_profile_hook
        sys.modules[name] = mod
        try:
            import antenv

            antenv.axon_hooks = mod
        except ImportError:
            pass
    mod = sys.modules[name]
    if mod.get_axon_ntff_profile_hook() is None:
        try:
            from trn_agent_boot.trn_boot import _ntff_profile_via_ctypes

            hook = _ntff_profile_via_ctypes("/opt/axon/libaxon_pjrt.so")
            if hook is not None:
                mod.set_axon_ntff_profile_hook(hook)
        except Exception:
            pass


_ensure_ntff_hook()

from concourse import bass, mybir, tile
from concourse.bass_utils import run_bass_kernel_spmd

F32 = mybir.dt.float32
BF16 = mybir.dt.bfloat16
ALU = mybir.AluOpType
ACTF = mybir.ActivationFunctionType
AX = mybir.AxisListType

B = 64
V = 5023
VM = 3500
LAT = 556
DIN = 3 * 224 * 224  # 150528
NCORES = 8
KSH = DIN // NCORES  # 18816
KT = KSH // 128  # 147 k-tiles
TPC = 7  # k-tiles per W chunk
NCH = KT // TPC  # 21 chunks
NCOLS = 411  # packed latent cols: 0:400 + 545:556
NOUT = 2 * VM + 68 + 11  # 7079
SL = 448  # verts per core (last core: 364 real + pad)
PL = SL + 68 + 4  # per-plane block: slice, fl, lme, rme, fc, vmean = 520
N2 = 3 * PL  # 1560
GAZE_DIR = -1.0
HALF_PI = 1.5707963267948966
# packed pose col offsets (orig 545:556 -> packed 400:411)
P_ROT, P_T, P_SC, P_LR, P_RR = 400, 403, 406, 407, 409


class Geo:
    """Helper for tiny per-batch scalar ops on [rows,1] tiles."""

    _uid = [0]

    def __init__(self, nc, pool, rows=B, eng=None):
        self.nc = nc
        self.pool = pool
        self.rows = rows
        self.eng = eng if eng is not None else nc.vector

    def t(self, cols=1):
        Geo._uid[0] += 1
        return self.pool.tile([self.rows, cols], F32, name=f"g{Geo._uid[0]}_{cols}")

    def mul(self, a, b):
        o = self.t()
        self.eng.tensor_tensor(out=o, in0=a, in1=b, op=ALU.mult)
        return o

    def add(self, a, b):
        o = self.t()
        self.eng.tensor_tensor(out=o, in0=a, in1=b, op=ALU.add)
        return o

    def sub(self, a, b):
        o = self.t()
        self.eng.tensor_tensor(out=o, in0=a, in1=b, op=ALU.subtract)
        return o

    def mac(self, a, s, acc):
        """(a * s) + acc, s is a [rows,1] AP scalar."""
        o = self.t()
        self.eng.scalar_tensor_tensor(
            out=o, in0=a, scalar=s, in1=acc, op0=ALU.mult, op1=ALU.add
        )
        return o

    def dot3(self, ax, ay, az, bx, by, bz):
        o = self.mul(ax, bx)
        o = self.mac(ay, by, o)
        o = self.mac(az, bz, o)
        return o

    def cross3(self, ax, ay, az, bx, by, bz):
        cx = self.sub(self.mul(ay, bz), self.mul(az, by))
        cy = self.sub(self.mul(az, bx), self.mul(ax, bz))
        cz = self.sub(self.mul(ax, by), self.mul(ay, bx))
        return cx, cy, cz


def axis_angle_R(nc, g, aa3, pfx, halfpi):
    R_ = g.rows
    """aa3: [rows,3] axis-angle tile -> R [rows,9] tile, R[l,i] at col l*3+i.

    GpSimd rejects tensor_scalar with AP scalar operands (TensorScalarPtr),
    so that path uses per-column tensor_tensor instead."""
    pool = g.pool
    eng = g.eng
    pool_safe = eng is nc.gpsimd

    def tsmul3(dst, src3, sap):
        if pool_safe:
            for j in range(3):
                eng.tensor_tensor(
                    out=dst[:, j:j + 1], in0=src3[:, j:j + 1], in1=sap, op=ALU.mult
                )
        else:
            eng.tensor_scalar_mul(out=dst, in0=src3, scalar1=sap)

    sq = pool.tile([R_, 3], F32, name=pfx + "aaR_sq")
    eng.tensor_tensor(out=sq, in0=aa3, in1=aa3, op=ALU.mult)
    th2a = g.t()
    eng.tensor_tensor(out=th2a, in0=sq[:, 0:1], in1=sq[:, 1:2], op=ALU.add)
    th2 = g.t()
    eng.tensor_tensor(out=th2, in0=th2a, in1=sq[:, 2:3], op=ALU.add)
    theta = g.t()
    nc.scalar.activation(out=theta, in_=th2, func=ACTF.Sqrt)
    thm = g.t()
    if pool_safe:
        eps = pool.tile([R_, 1], F32, name=pfx + "aaR_eps")
        eng.memset(eps, 1e-8)
        eng.tensor_tensor(out=thm, in0=theta, in1=eps, op=ALU.max)
    else:
        eng.tensor_scalar_max(out=thm, in0=theta, scalar1=1e-8)
    rth = g.t()
    nc.vector.reciprocal(out=rth, in_=thm)
    axis3 = pool.tile([R_, 3], F32, name=pfx + "aaR_axis")
    tsmul3(axis3, aa3, rth)
    s = g.t()
    nc.scalar.activation(out=s, in_=theta, func=ACTF.Sin)
    c = g.t()
    nc.scalar.activation(out=c, in_=theta, func=ACTF.Sin, bias=halfpi)
    omc = g.t()
    if pool_safe:
        one_t = pool.tile([R_, 1], F32, name=pfx + "aaR_one")
        eng.memset(one_t, 1.0)
        eng.tensor_tensor(out=omc, in0=one_t, in1=c, op=ALU.subtract)
    else:
        eng.tensor_scalar(
            out=omc, in0=c, scalar1=-1.0, scalar2=1.0, op0=ALU.mult, op1=ALU.add
        )
    ax, ay, az = axis3[:, 0:1], axis3[:, 1:2], axis3[:, 2:3]
    asq = pool.tile([R_, 3], F32, name=pfx + "aaR_asq")
    eng.tensor_tensor(out=asq, in0=axis3, in1=axis3, op=ALU.mult)
    R = pool.tile([R_, 9], F32, name=pfx + "aaR_R")
    dmul = pool.tile([R_, 3], F32, name=pfx + "aaR_dmul")
    tsmul3(dmul, asq, omc)
    sa = pool.tile([R_, 3], F32, name=pfx + "aaR_sa")
    tsmul3(sa, axis3, s)
    sax, say, saz = sa[:, 0:1], sa[:, 1:2], sa[:, 2:3]
    mxy = g.mul(g.mul(ax, ay), omc)
    mxz = g.mul(g.mul(ax, az), omc)
    myz = g.mul(g.mul(ay, az), omc)
    for l in range(3):
        eng.tensor_tensor(
            out=R[:, 4 * l:4 * l + 1], in0=dmul[:, l:l + 1], in1=c, op=ALU.add
        )
    eng.tensor_tensor(out=R[:, 1:2], in0=mxy, in1=saz, op=ALU.subtract)  # R01
    eng.tensor_tensor(out=R[:, 2:3], in0=mxz, in1=say, op=ALU.add)  # R02
    eng.tensor_tensor(out=R[:, 3:4], in0=mxy, in1=saz, op=ALU.add)  # R10
    eng.tensor_tensor(out=R[:, 5:6], in0=myz, in1=sax, op=ALU.subtract)  # R12
    eng.tensor_tensor(out=R[:, 6:7], in0=mxz, in1=say, op=ALU.subtract)  # R20
    eng.tensor_tensor(out=R[:, 7:8], in0=myz, in1=sax, op=ALU.add)  # R21
    return R


_ENG_ATTR = {
    "SP": "sync", "Pool": "gpsimd", "PE": "tensor",
    "DVE": "vector", "Activation": "scalar",
}


def _legalize_waits(nc):
    """This walrus accepts only one sync-wait slot per instruction; move extra
    waits onto same-engine NoOps inserted right before the instruction."""
    import concourse.mybir as _mybir

    def make_nop(engine):
        eng = getattr(nc, _ENG_ATTR[engine.name])
        bi = eng.nop(nofuse=True)
        mi = bi.ins
        for bb in nc.main_func.blocks:
            if bb.instructions and bb.instructions[-1].name == mi.name:
                bb.instructions.pop()
                break
        mi.engine = engine
        return mi

    for bb in nc.main_func.blocks:
        snapshot = list(bb.instructions)
        newlist = []
        changed = False
        for inst in snapshot:
            si = inst.sync_info
            waits = list(si.on_wait) if (si and si.on_wait) else []
            if (
                len(waits) > 1
                and not inst.name.startswith("barrier")
                and inst.engine is not None
                and getattr(inst.engine, "name", None) in _ENG_ATTR
            ):
                for w in waits[:-1]:
                    nop = make_nop(inst.engine)
                    nop.sync_info = _mybir.SyncInfo(on_wait=[w], on_update=[])
                    newlist.append(nop)
                inst.sync_info = _mybir.SyncInfo(
                    on_wait=[waits[-1]], on_update=list(si.on_update)
                )
                changed = True
            newlist.append(inst)
        if changed:
            bb.instructions[:] = newlist
    return nc


XPARTS = [25, 25, 25, 25, 25, 22]  # k-tile split of the resident x shard


def build_graph():
    nc = bass.Bass(target_bir_lowering=False)

    x_p = nc.declare_dram_parameter("xw", [128, KT, 2, B], BF16, isOutput=False)
    w_p = nc.declare_dram_parameter(
        "wch", [NCH, 128, TPC, 2, NCOLS], BF16, isOutput=False
    )
    b_p = nc.declare_dram_parameter("bvec", [1, NCOLS + 1], F32, isOutput=False)
    bas_p = nc.declare_dram_parameter("basis", [128, 4, N2], F32, isOutput=False)
    cam_p = nc.declare_dram_parameter("cam", [B, 12], F32, isOutput=False)
    eye_p = nc.declare_dram_parameter("eye", [B, B], F32, isOutput=False)
    out_p = nc.declare_dram_parameter("out", [B, 3, 2 * SL + 71 + 8], F32, isOutput=True)

    ar_in = nc.dram_tensor("ar_in", [B, NCOLS + 1], F32)
    ar_out = nc.dram_tensor("ar_out", [B, NCOLS + 1], F32, addr_space="Shared")

    with tile.TileContext(nc) as tc:
        with (
            tc.tile_pool(name="consts", bufs=1) as consts,
            tc.tile_pool(name="xres", bufs=1) as xres,
            tc.tile_pool(name="latp", bufs=1) as latp,
            tc.tile_pool(name="geop", bufs=1) as geop,
            tc.tile_pool(name="planep", bufs=1) as planep,
            tc.tile_pool(name="dum", bufs=1, space="PSUM") as dum,
        ):
            # ---- const / prefetch loads ----
            eye_sb = consts.tile([B, B], F32)
            nc.scalar.dma_start(out=eye_sb, in_=eye_p[:, :])
            cam = consts.tile([B, 12], F32)
            nc.scalar.dma_start(out=cam, in_=cam_p[:, :])
            b_sb = consts.tile([1, NCOLS + 1], F32)
            nc.scalar.dma_start(out=b_sb, in_=b_p[:, :])
            ones1 = consts.tile([1, B], F32)
            nc.vector.memset(ones1, 1.0)
            halfpi = consts.tile([128, 1], F32)
            nc.vector.memset(halfpi, HALF_PI)
            lat = latp.tile([B, 416], F32)
            nc.vector.memset(lat, 0.0)

            # resident x shard (hi+lo), split so PE can start after part 0
            xts = []
            off = 0
            for pi, n in enumerate(XPARTS):
                xt = xres.tile([128, n, 2, B], BF16, name=f"xt{pi}")
                nc.gpsimd.dma_start(out=xt, in_=x_p[:, off:off + n, :, :])
                xts.append((off, n, xt))
                off += n
            # phase-2 basis block (prefetch; lands during phase 1)
            basis_sb = planep.tile([128, 4, N2], F32)
            nc.gpsimd.dma_start(out=basis_sb, in_=bas_p[:, :, :])

            def xap(k, hl):
                for off, n, xt in xts:
                    if k < off + n:
                        return xt[:, k - off, hl, :]
                raise IndexError(k)

            d1 = dum.tile([1, 1], F32)

            # ---------------- Phase 1: encoder GEMM (hi/lo bf16) ----------------
            with (
                tc.tile_pool(name="wts", bufs=6) as wts,
                tc.tile_pool(name="encp", bufs=1, space="PSUM") as encp,
            ):
                pe = encp.tile([B, NCOLS + 1], F32)
                nc.tensor.matmul(
                    d1, lhsT=xts[0][2][:, 0, 0, 0:1], rhs=xts[0][2][:, 0, 0, 0:1],
                    start=True, stop=True, skip_group_check=True,
                )
                for ci in range(NCH):
                    w_c = wts.tile([128, TPC, 2, NCOLS], BF16)
                    eng = nc.sync if ci % 2 == 0 else nc.scalar
                    eng.dma_start(out=w_c, in_=w_p[ci])
                    for t in range(TPC):
                        k = ci * TPC + t
                        nc.tensor.matmul(
                            pe[:, 0:NCOLS], lhsT=xap(k, 0), rhs=w_c[:, t, 0, :],
                            start=(k == 0), stop=False,
                        )
                        nc.tensor.matmul(
                            pe[:, 0:NCOLS], lhsT=xap(k, 0), rhs=w_c[:, t, 1, :],
                            start=False, stop=False,
                        )
                        nc.tensor.matmul(
                            pe[:, 0:NCOLS], lhsT=xap(k, 1), rhs=w_c[:, t, 0, :],
                            start=False, stop=False,
                        )
                # bias (scaled 1/8) + constant 1/8 lane in col 411, fp32
                nc.tensor.matmul(
                    d1, lhsT=b_sb[0:1, 0:1], rhs=b_sb[0:1, 0:1],
                    start=True, stop=True, skip_group_check=True,
                )
                nc.tensor.matmul(
                    pe, lhsT=ones1, rhs=b_sb, start=False, stop=True,
                )
                lat1 = latp.tile([B, NCOLS + 1], F32)
                nc.vector.tensor_copy(out=lat1, in_=pe)
                nc.sync.dma_start(out=ar_in[:, :], in_=lat1)

            nc.gpsimd.collective_compute(
                "AllReduce",
                ALU.add,
                replica_groups=[list(range(NCORES))],
                ins=[ar_in.ap().opt()],
                outs=[ar_out.ap().opt()],
            )
            nc.sync.dma_start(out=lat[:, 0:NCOLS + 1], in_=ar_out[:, :])

            # ---------------- Phase 1.5: transpose shape params on PE ----------
            with tc.tile_pool(name="trps", bufs=1, space="PSUM") as trps:
                trp = trps.tile([128, 4, B], F32)
                nc.tensor.matmul(
                    d1, lhsT=eye_sb[0:1, 0:1], rhs=eye_sb[0:1, 0:1],
                    start=True, stop=True, skip_group_check=True,
                )
                for kt in range(3):
                    nc.tensor.matmul(
                        trp[:, kt, :], lhsT=lat[:, kt * 128:(kt + 1) * 128],
                        rhs=eye_sb, is_transpose=True,
                        start=True, stop=True, skip_group_check=True,
                    )
                nc.tensor.matmul(
                    trp[0:32, 3, :], lhsT=lat[:, 384:416],
                    rhs=eye_sb, is_transpose=True,
                    start=True, stop=True, skip_group_check=True,
                )
                spT = latp.tile([128, 4, B], F32)
                nc.scalar.copy(out=spT, in_=trp)

            # ---------------- Phase 2: blendshape GEMM (V-sharded) -------------
            vpre = planep.tile([B, N2], F32)
            NSPL2 = [(0, 512), (512, 512), (1024, 512), (1536, N2 - 1536)]
            with tc.tile_pool(name="p2ps", bufs=1, space="PSUM") as p2ps:
                pvs = [
                    p2ps.tile([B, n], F32, name=f"pv{j}", tag=f"pv{j}")
                    for j, (_, n) in enumerate(NSPL2)
                ]
                nc.tensor.matmul(
                    d1, lhsT=basis_sb[0:1, 0, 0:1], rhs=basis_sb[0:1, 0, 0:1],
                    start=True, stop=True, skip_group_check=True,
                )
                for j, (n0, n) in enumerate(NSPL2):
                    for kt in range(4):
                        rows = 128 if kt < 3 else 32
                        nc.tensor.matmul(
                            pvs[j],
                            lhsT=spT[0:rows, kt, :],
                            rhs=basis_sb[0:rows, kt, n0:n0 + n],
                            start=(kt == 0),
                            stop=(kt == 3),
                        )

                # eyeball rotation inputs (copies allowed on GpSimd)
                aa2 = geop.tile([128, 3], F32)
                nc.gpsimd.memset(aa2, 0.0)
                nc.gpsimd.tensor_copy(out=aa2[0:B, 0:2], in_=lat[:, P_LR:P_LR + 2])
                nc.sync.dma_start(out=aa2[B:128, 0:2], in_=lat[:, P_RR:P_RR + 2])

                # face rotation (DVE, overlaps the GEMM)
                g = Geo(nc, geop)
                Rf = axis_angle_R(nc, g, lat[:, P_ROT:P_ROT + 3], "f_", halfpi[:B, :])
                fs = g.t()
                nc.vector.tensor_scalar_add(
                    out=fs, in0=lat[:, P_SC:P_SC + 1], scalar1=1.0
                )
                Rs = geop.tile([B, 9], F32)
                nc.vector.tensor_scalar_mul(out=Rs, in0=Rf, scalar1=fs)

                for j, (n0, n) in enumerate([NSPL2[3], NSPL2[1], NSPL2[2], NSPL2[0]]):
                    nc.scalar.copy(out=vpre[:, n0:n0 + n], in_=pvs[NSPL2.index((n0, n))])

            # offsets: off_i = face_t_i - sum_l vms_l*Rs[l,i]
            off3 = geop.tile([B, 3], F32)
            for i in range(3):
                t = g.mul(vpre[:, 519:520], Rs[:, i:i + 1])
                t = g.mac(vpre[:, 520 + 519:520 + 520], Rs[:, 3 + i:4 + i], t)
                t = g.mac(vpre[:, 1040 + 519:1040 + 520], Rs[:, 6 + i:7 + i], t)
                nc.vector.tensor_tensor(
                    out=off3[:, i:i + 1], in0=lat[:, P_T + i:P_T + i + 1], in1=t,
                    op=ALU.subtract,
                )

            # rotate + translate all plane blocks
            rt = planep.tile([B, 3, PL], F32)
            for i in range(3):
                nc.vector.tensor_scalar(
                    out=rt[:, i, :], in0=vpre[:, 0:PL],
                    scalar1=Rs[:, i:i + 1], scalar2=off3[:, i:i + 1],
                    op0=ALU.mult, op1=ALU.add,
                )
                for l in (1, 2):
                    nc.vector.scalar_tensor_tensor(
                        out=rt[:, i, :], in0=vpre[:, l * PL:(l + 1) * PL],
                        scalar=Rs[:, 3 * l + i:3 * l + i + 1],
                        in1=rt[:, i, :],
                        op0=ALU.mult, op1=ALU.add,
                    )

            lc = [rt[:, i, SL + 68:SL + 69] for i in range(3)]
            rc = [rt[:, i, SL + 69:SL + 70] for i in range(3)]

            # projection of this core's vert slice (DVE)
            with tc.tile_pool(name="imgp", bufs=1) as imgp:
                img = imgp.tile([B, 3, SL], F32)
                for i in (2, 0, 1):  # z first (feeds the clamp chain)
                    nc.vector.tensor_scalar(
                        out=img[:, i, :], in0=rt[:, 0, 0:SL],
                        scalar1=cam[:, 4 * i:4 * i + 1],
                        scalar2=cam[:, 4 * i + 3:4 * i + 4],
                        op0=ALU.mult, op1=ALU.add,
                    )
                    for l in (1, 2):
                        nc.vector.scalar_tensor_tensor(
                            out=img[:, i, :], in0=rt[:, l, 0:SL],
                            scalar=cam[:, 4 * i + l:4 * i + l + 1], in1=img[:, i, :],
                            op0=ALU.mult, op1=ALU.add,
                        )
                az_ = imgp.tile([B, SL], F32)
                nc.scalar.activation(out=az_, in_=img[:, 2, :], func=ACTF.Abs)
                nc.vector.tensor_scalar_max(out=az_, in0=az_, scalar1=1e-3)
                sg = imgp.tile([B, SL], F32)
                nc.vector.tensor_scalar(
                    out=sg, in0=img[:, 2, :], scalar1=0.0, scalar2=None, op0=ALU.is_ge
                )
                nc.vector.tensor_scalar(
                    out=sg, in0=sg, scalar1=2.0, scalar2=1.0,
                    op0=ALU.mult, op1=ALU.subtract,
                )
                nc.vector.tensor_tensor(out=sg, in0=sg, in1=az_, op=ALU.mult)
                nc.vector.reciprocal(out=az_, in_=sg)
                nc.vector.tensor_tensor(
                    out=img[:, 0, :], in0=img[:, 0, :], in1=az_, op=ALU.mult
                )
                nc.vector.tensor_tensor(
                    out=img[:, 1, :], in0=img[:, 1, :], in1=az_, op=ALU.mult
                )

                # eyeball rotations (DVE tail; only gates the Cramer solve)
                g2 = Geo(nc, geop, rows=128)
                R2 = axis_angle_R(nc, g2, aa2, "e_", halfpi)
                gz = geop.tile([128, 3], F32)
                nc.vector.tensor_scalar_mul(out=gz, in0=R2[:, 6:9], scalar1=GAZE_DIR)
                rg64 = geop.tile([B, 3], F32)
                nc.sync.dma_start(out=rg64, in_=gz[B:128, :])
                lg = [gz[0:B, i:i + 1] for i in range(3)]
                rg = [rg64[:, i:i + 1] for i in range(3)]

                # vert + img outputs can ship while the tail is computed
                nc.sync.dma_start(out=out_p[:, :, 0:SL], in_=rt[:, :, 0:SL])
                nc.scalar.dma_start(out=out_p[:, :, SL:2 * SL], in_=img)
                nc.sync.dma_start(
                    out=out_p[:, :, 2 * SL:2 * SL + 71], in_=rt[:, :, SL:SL + 71]
                )

                # tail block ge[:, i, j]: gp_l gp_r gp_mid far_l far_r lg rg dist
                ge = geop.tile([B, 3, 8], F32)
                for i in range(3):
                    # independent pieces off the DVE critical chain
                    nc.vector.scalar_tensor_tensor(
                        out=ge[:, i, 3:4], in0=lg[i], scalar=1000.0,
                        in1=lc[i], op0=ALU.mult, op1=ALU.add,
                    )
                    nc.vector.scalar_tensor_tensor(
                        out=ge[:, i, 4:5], in0=rg[i], scalar=1000.0,
                        in1=rc[i], op0=ALU.mult, op1=ALU.add,
                    )
                    nc.gpsimd.tensor_copy(out=ge[:, i, 5:6], in_=lg[i])
                    nc.gpsimd.tensor_copy(out=ge[:, i, 6:7], in_=rg[i])

                # gaze intersection (Cramer, DVE)
                d = [g.sub(rc[i], lc[i]) for i in range(3)]
                c1 = []
                for i in range(3):
                    o = g.t()
                    nc.vector.tensor_scalar_mul(out=o, in0=rg[i], scalar1=-1.0)
                    c1.append(o)
                c2 = list(g.cross3(*rg, *lg))
                w = g.cross3(*c1, *c2)
                det = g.dot3(*lg, *w)
                num0 = g.dot3(*d, *w)
                w2 = g.cross3(*d, *c2)
                num1 = g.dot3(*lg, *w2)
                rdet = g.t()
                nc.vector.reciprocal(out=rdet, in_=det)
                sol0 = g.mul(num0, rdet)
                sol1 = g.mul(num1, rdet)

                gpl = geop.tile([B, 3], F32)
                gpr = geop.tile([B, 3], F32)
                for i in range(3):
                    nc.vector.scalar_tensor_tensor(
                        out=gpl[:, i:i + 1], in0=lg[i], scalar=sol0,
                        in1=lc[i], op0=ALU.mult, op1=ALU.add,
                    )
                    nc.vector.scalar_tensor_tensor(
                        out=gpr[:, i:i + 1], in0=rg[i], scalar=sol1,
                        in1=rc[i], op0=ALU.mult, op1=ALU.add,
                    )
                    nc.vector.tensor_copy(out=ge[:, i, 0:1], in_=gpl[:, i:i + 1])
                    nc.vector.tensor_copy(out=ge[:, i, 1:2], in_=gpr[:, i:i + 1])
                    o = g.add(gpl[:, i:i + 1], gpr[:, i:i + 1])
                    nc.vector.tensor_scalar_mul(out=ge[:, i, 2:3], in0=o, scalar1=0.5)
                dff = geop.tile([B, 3], F32)
                nc.vector.tensor_tensor(out=dff, in0=gpl, in1=gpr, op=ALU.subtract)
                nc.vector.tensor_tensor(out=dff, in0=dff, in1=dff, op=ALU.mult)
                d2 = g.t()
                nc.vector.tensor_reduce(out=d2, in_=dff, axis=AX.X, op=ALU.add)
                dist = g.t()
                nc.scalar.activation(out=dist, in_=d2, func=ACTF.Sqrt)
                for i in range(3):
                    nc.scalar.copy(out=ge[:, i, 7:8], in_=dist)

                nc.scalar.dma_start(out=out_p[:, :, 2 * SL + 71:2 * SL + 79], in_=ge)
    _legalize_waits(nc)
    return nc


def _prep(inputs):
    f32 = np.float32
    x = np.ascontiguousarray(inputs["x"].reshape(B, DIN), dtype=f32)
    W = np.asarray(inputs["enc_W"], dtype=f32)
    Wp = np.concatenate([W[:, :400], W[:, 545:556]], axis=1)  # [DIN, 411]
    enc_b = np.asarray(inputs["enc_b"], dtype=f32)
    bp = np.concatenate([enc_b[:400], enc_b[545:556]])
    bvec = np.concatenate(
        [bp / NCORES, np.array([1.0 / NCORES], f32)]
    ).reshape(1, NCOLS + 1).astype(f32)
    tmpl = np.asarray(inputs["v_template"], dtype=f32)  # [V, 3]
    basis = np.asarray(inputs["shape_basis"], dtype=f32)  # [400, V, 3]
    cam = np.ascontiguousarray(
        np.asarray(inputs["camera_parameters"], dtype=f32).reshape(B, 12)
    )
    lm = np.asarray(inputs["landmarks"])
    mlm = np.asarray(inputs["masked_landmarks"])
    fmask = np.asarray(inputs["face_mask"])
    lmask = np.asarray(inputs["left_eyeball_mask"])
    rmask = np.asarray(inputs["right_eyeball_mask"])
    fl_idx = fmask[mlm]  # verts behind the 68 output landmarks
    idx4 = lm[np.array([19, 22, 25, 28])]
    idx2 = lm[np.array([14, 18])]

    # synthetic extra columns [400, 72, 3] / [72, 3]
    ex_b = np.concatenate([
        basis[:, fl_idx, :],
        basis[:, lmask, :].mean(axis=1, keepdims=True),
        basis[:, rmask, :].mean(axis=1, keepdims=True),
        (basis[:, idx4, :].mean(axis=1, keepdims=True)
         + basis[:, idx2, :].mean(axis=1, keepdims=True)) / 2.0,
        basis.mean(axis=1, keepdims=True),
    ], axis=1)
    ex_t = np.concatenate([
        tmpl[fl_idx],
        tmpl[lmask].mean(axis=0, keepdims=True),
        tmpl[rmask].mean(axis=0, keepdims=True),
        (tmpl[idx4].mean(axis=0, keepdims=True)
         + tmpl[idx2].mean(axis=0, keepdims=True)) / 2.0,
        tmpl.mean(axis=0, keepdims=True),
    ], axis=0)

    eye = np.eye(B, dtype=f32)
    in_maps = []
    for c in range(NCORES):
        k0 = c * KSH
        xs = x[:, k0:k0 + KSH].T  # [KSH, B] f32
        xh = xs.astype(BF)
        xl = (xs - xh.astype(f32)).astype(BF)
        xw = np.ascontiguousarray(
            np.stack([
                xh.reshape(KT, 128, B).transpose(1, 0, 2),
                xl.reshape(KT, 128, B).transpose(1, 0, 2),
            ], axis=2)
        )  # [128, KT, 2, B] bf16
        ws = Wp[k0:k0 + KSH]  # [KSH, 411] f32
        wh = ws.astype(BF)
        wl = (ws - wh.astype(f32)).astype(BF)
        wch = np.ascontiguousarray(
            np.stack([
                wh.reshape(NCH, TPC, 128, NCOLS).transpose(0, 2, 1, 3),
                wl.reshape(NCH, TPC, 128, NCOLS).transpose(0, 2, 1, 3),
            ], axis=3)
        )  # [NCH, 128, TPC, 2, 411] bf16

        lo = c * SL
        verts = fmask[lo:min(lo + SL, VM)]
        nsl = len(verts)
        blk = np.zeros((400, N2), f32)
        trow = np.zeros(N2, f32)
        for l in range(3):
            blk[:, l * PL:l * PL + nsl] = basis[:, verts, l]
            blk[:, l * PL + SL:l * PL + SL + 72] = ex_b[:, :, l]
            trow[l * PL:l * PL + nsl] = tmpl[verts, l]
            trow[l * PL + SL:l * PL + SL + 72] = ex_t[:, l]
        bh = np.zeros((128, 4, N2), f32)
        for kt in range(3):
            bh[:, kt, :] = blk[kt * 128:(kt + 1) * 128]
        bh[0:16, 3, :] = blk[384:400]
        bh[27, 3, :] = trow  # coefficient = exact 1.0 from AR col 411
        in_maps.append({
            "xw": xw,
            "wch": wch,
            "bvec": bvec,
            "basis": np.ascontiguousarray(bh),
            "cam": cam,
            "eye": eye,
        })
    return in_maps


def _run(inputs, trace=False):
    in_maps = _prep(inputs)
    nc = build_graph()
    res = run_bass_kernel_spmd(
        nc, in_maps, core_ids=list(range(NCORES)), trace=trace
    )
    full = np.empty((B, 3, NOUT), np.float32)
    for c in range(NCORES):
        r = res.results[c]["out"]  # [B, 3, 975]
        lo = c * SL
        w = min(SL, VM - lo)
        full[:, :, lo:lo + w] = r[:, :, 0:w]
        full[:, :, VM + lo:VM + lo + w] = r[:, :, SL:SL + w]
    r0 = res.results[0]["out"]
    full[:, :, 2 * VM:NOUT] = r0[:, :, 2 * SL:2 * SL + 79]
    return np.ascontiguousarray(full.transpose(0, 2, 1)), res


def kernel(**inputs):
    out, _ = _run(inputs, trace=False)
    return out


# revision 12
# speedup vs baseline: 2.1619x; 1.0765x over previous
"""Trainium2 Bass kernel for nn_Autoencoder_65223373357102 (FLAME-style autoencoder).

Strategy (v3):
  Phase 1 (8-way tensor parallel): encoder GEMM sharded along K, W packed to
  the 411 *used* latent columns. The fp32 GEMM is decomposed into three bf16
  passes (x_hi*W_hi + x_hi*W_lo + x_lo*W_hi, fp32 PSUM accumulation): bf16
  products are exact in fp32, so the latent error is ~4e-6 relative - inside
  the ~1e-5 budget set by the z-clamped projection - while the PE runs 1
  cycle/row instead of fp32's 4. x (hi+lo) is SBUF-resident; W streams in 21
  pre-tiled contiguous chunks on two DMA queues, deep-buffered so the NRT
  start barrier overlaps prefetch. Bias (scaled 1/8) and a constant 1/8 lane
  (col 411) are folded into the PSUM accumulation; the AllReduce of [64,412]
  then yields latent + an exact 1.0 in col 411 that phase 2 uses as the
  template coefficient.
  Phase 2 (8-way vertex parallel): each core computes only its 448 of the 3500
  face verts plus 72 synthetic columns (68 landmarks, l/r eye means, face
  centre, vmean) via an fp32 [64,400+]@[400+,1560] GEMM from host-gathered
  basis columns. Everything the reference does to the eye vertex slices is
  dead code w.r.t. the output (only the eye means and gaze rotations survive).
  shape_p is transposed on the PE (identity matmul). The eyeball-rotation
  chain runs on GpSimd in parallel with the DVE rotate/project chain. Per-core
  output [64,3,975] is stitched to the full [64,7079,3] on the host.
"""
import sys
import types

sys.path.insert(0, "/opt/trn_rl_repo")

import numpy as np
import ml_dtypes

BF = ml_dtypes.bfloat16


def _ensure_ntff_hook():
    """Provide antenv.axon_hooks + install the ctypes NTFF profile hook so
    run_bass_kernel_spmd(trace=True) can pull a neuron-profile under axon."""
    name = "antenv.axon_hooks"
    if name not in sys.modules:
        mod = types.ModuleType(name)
        mod._HOOK = None

        def set_axon_ntff_profile_hook(hook):
            mod._HOOK = hook

        def get_axon_ntff_profile_hook():
            return mod._HOOK

        mod.set_axon_ntff_profile_hook = set_axon_ntff_profile_hook
        mod.get_axon_ntff_profile_hook = get_axon_ntff_profile_hook
        sys.modules[name] = mod
        try:
            import antenv

            antenv.axon_hooks = mod
        except ImportError:
            pass
    mod = sys.modules[name]
    if mod.get_axon_ntff_profile_hook() is None:
        try:
            from trn_agent_boot.trn_boot import _ntff_profile_via_ctypes

            hook = _ntff_profile_via_ctypes("/opt/axon/libaxon_pjrt.so")
            if hook is not None:
                mod.set_axon_ntff_profile_hook(hook)
        except Exception:
            pass


_ensure_ntff_hook()

from concourse import bass, mybir, tile
from concourse.bass_utils import run_bass_kernel_spmd

F32 = mybir.dt.float32
BF16 = mybir.dt.bfloat16
ALU = mybir.AluOpType
ACTF = mybir.ActivationFunctionType
AX = mybir.AxisListType

B = 64
V = 5023
VM = 3500
LAT = 556
DIN = 3 * 224 * 224  # 150528
NCORES = 8
KSH = DIN // NCORES  # 18816
KT = KSH // 128  # 147 k-tiles
TPC = 7  # k-tiles per W chunk
NCH = KT // TPC  # 21 chunks
NCOLS = 411  # packed latent cols: 0:400 + 545:556
NOUT = 2 * VM + 68 + 11  # 7079
SL = 448  # verts per core (last core: 364 real + pad)
PL = SL + 68 + 4  # per-plane block: slice, fl, lme, rme, fc, vmean = 520
N2 = 3 * PL  # 1560
GAZE_DIR = -1.0
HALF_PI = 1.5707963267948966
# packed pose col offsets (orig 545:556 -> packed 400:411)
P_ROT, P_T, P_SC, P_LR, P_RR = 400, 403, 406, 407, 409


class Geo:
    """Helper for tiny per-batch scalar ops on [rows,1] tiles."""

    _uid = [0]

    def __init__(self, nc, pool, rows=B, eng=None):
        self.nc = nc
        self.pool = pool
        self.rows = rows
        self.eng = eng if eng is not None else nc.vector

    def t(self, cols=1):
        Geo._uid[0] += 1
        return self.pool.tile([self.rows, cols], F32, name=f"g{Geo._uid[0]}_{cols}")

    def mul(self, a, b):
        o = self.t()
        self.eng.tensor_tensor(out=o, in0=a, in1=b, op=ALU.mult)
        return o

    def add(self, a, b):
        o = self.t()
        self.eng.tensor_tensor(out=o, in0=a, in1=b, op=ALU.add)
        return o

    def sub(self, a, b):
        o = self.t()
        self.eng.tensor_tensor(out=o, in0=a, in1=b, op=ALU.subtract)
        return o

    def mac(self, a, s, acc):
        """(a * s) + acc, s is a [rows,1] AP scalar."""
        o = self.t()
        self.eng.scalar_tensor_tensor(
            out=o, in0=a, scalar=s, in1=acc, op0=ALU.mult, op1=ALU.add
        )
        return o

    def dot3(self, ax, ay, az, bx, by, bz):
        o = self.mul(ax, bx)
        o = self.mac(ay, by, o)
        o = self.mac(az, bz, o)
        return o

    def cross3(self, ax, ay, az, bx, by, bz):
        cx = self.sub(self.mul(ay, bz), self.mul(az, by))
        cy = self.sub(self.mul(az, bx), self.mul(ax, bz))
        cz = self.sub(self.mul(ax, by), self.mul(ay, bx))
        return cx, cy, cz


def axis_angle_R(nc, g, aa3, pfx, halfpi):
    R_ = g.rows
    """aa3: [rows,3] axis-angle tile -> R [rows,9] tile, R[l,i] at col l*3+i.

    GpSimd rejects tensor_scalar with AP scalar operands (TensorScalarPtr),
    so that path uses per-column tensor_tensor instead."""
    pool = g.pool
    eng = g.eng
    pool_safe = eng is nc.gpsimd

    def tsmul3(dst, src3, sap):
        if pool_safe:
            for j in range(3):
                eng.tensor_tensor(
                    out=dst[:, j:j + 1], in0=src3[:, j:j + 1], in1=sap, op=ALU.mult
                )
        else:
            eng.tensor_scalar_mul(out=dst, in0=src3, scalar1=sap)

    sq = pool.tile([R_, 3], F32, name=pfx + "aaR_sq")
    eng.tensor_tensor(out=sq, in0=aa3, in1=aa3, op=ALU.mult)
    th2a = g.t()
    eng.tensor_tensor(out=th2a, in0=sq[:, 0:1], in1=sq[:, 1:2], op=ALU.add)
    th2 = g.t()
    eng.tensor_tensor(out=th2, in0=th2a, in1=sq[:, 2:3], op=ALU.add)
    theta = g.t()
    nc.scalar.activation(out=theta, in_=th2, func=ACTF.Sqrt)
    thm = g.t()
    if pool_safe:
        eps = pool.tile([R_, 1], F32, name=pfx + "aaR_eps")
        eng.memset(eps, 1e-8)
        eng.tensor_tensor(out=thm, in0=theta, in1=eps, op=ALU.max)
    else:
        eng.tensor_scalar_max(out=thm, in0=theta, scalar1=1e-8)
    rth = g.t()
    nc.vector.reciprocal(out=rth, in_=thm)
    axis3 = pool.tile([R_, 3], F32, name=pfx + "aaR_axis")
    tsmul3(axis3, aa3, rth)
    s = g.t()
    nc.scalar.activation(out=s, in_=theta, func=ACTF.Sin)
    c = g.t()
    nc.scalar.activation(out=c, in_=theta, func=ACTF.Sin, bias=halfpi)
    omc = g.t()
    if pool_safe:
        one_t = pool.tile([R_, 1], F32, name=pfx + "aaR_one")
        eng.memset(one_t, 1.0)
        eng.tensor_tensor(out=omc, in0=one_t, in1=c, op=ALU.subtract)
    else:
        eng.tensor_scalar(
            out=omc, in0=c, scalar1=-1.0, scalar2=1.0, op0=ALU.mult, op1=ALU.add
        )
    ax, ay, az = axis3[:, 0:1], axis3[:, 1:2], axis3[:, 2:3]
    asq = pool.tile([R_, 3], F32, name=pfx + "aaR_asq")
    eng.tensor_tensor(out=asq, in0=axis3, in1=axis3, op=ALU.mult)
    R = pool.tile([R_, 9], F32, name=pfx + "aaR_R")
    dmul = pool.tile([R_, 3], F32, name=pfx + "aaR_dmul")
    tsmul3(dmul, asq, omc)
    sa = pool.tile([R_, 3], F32, name=pfx + "aaR_sa")
    tsmul3(sa, axis3, s)
    sax, say, saz = sa[:, 0:1], sa[:, 1:2], sa[:, 2:3]
    mxy = g.mul(g.mul(ax, ay), omc)
    mxz = g.mul(g.mul(ax, az), omc)
    myz = g.mul(g.mul(ay, az), omc)
    for l in range(3):
        eng.tensor_tensor(
            out=R[:, 4 * l:4 * l + 1], in0=dmul[:, l:l + 1], in1=c, op=ALU.add
        )
    eng.tensor_tensor(out=R[:, 1:2], in0=mxy, in1=saz, op=ALU.subtract)  # R01
    eng.tensor_tensor(out=R[:, 2:3], in0=mxz, in1=say, op=ALU.add)  # R02
    eng.tensor_tensor(out=R[:, 3:4], in0=mxy, in1=saz, op=ALU.add)  # R10
    eng.tensor_tensor(out=R[:, 5:6], in0=myz, in1=sax, op=ALU.subtract)  # R12
    eng.tensor_tensor(out=R[:, 6:7], in0=mxz, in1=say, op=ALU.subtract)  # R20
    eng.tensor_tensor(out=R[:, 7:8], in0=myz, in1=sax, op=ALU.add)  # R21
    return R


_ENG_ATTR = {
    "SP": "sync", "Pool": "gpsimd", "PE": "tensor",
    "DVE": "vector", "Activation": "scalar",
}


def _legalize_waits(nc):
    """This walrus accepts only one sync-wait slot per instruction; move extra
    waits onto same-engine NoOps inserted right before the instruction."""
    import concourse.mybir as _mybir

    def make_nop(engine):
        eng = getattr(nc, _ENG_ATTR[engine.name])
        bi = eng.nop(nofuse=True)
        mi = bi.ins
        for bb in nc.main_func.blocks:
            if bb.instructions and bb.instructions[-1].name == mi.name:
                bb.instructions.pop()
                break
        mi.engine = engine
        return mi

    for bb in nc.main_func.blocks:
        snapshot = list(bb.instructions)
        newlist = []
        changed = False
        for inst in snapshot:
            si = inst.sync_info
            waits = list(si.on_wait) if (si and si.on_wait) else []
            if (
                len(waits) > 1
                and not inst.name.startswith("barrier")
                and inst.engine is not None
                and getattr(inst.engine, "name", None) in _ENG_ATTR
            ):
                for w in waits[:-1]:
                    nop = make_nop(inst.engine)
                    nop.sync_info = _mybir.SyncInfo(on_wait=[w], on_update=[])
                    newlist.append(nop)
                inst.sync_info = _mybir.SyncInfo(
                    on_wait=[waits[-1]], on_update=list(si.on_update)
                )
                changed = True
            newlist.append(inst)
        if changed:
            bb.instructions[:] = newlist
    return nc


XPARTS = [25, 25, 25, 25, 25, 22]  # k-tile split of the resident x shard


def build_graph():
    nc = bass.Bass(target_bir_lowering=False)

    x_p = nc.declare_dram_parameter("xw", [128, KT, 2, B], BF16, isOutput=False)
    w_p = nc.declare_dram_parameter(
        "wch", [NCH, 128, TPC, 2, NCOLS], BF16, isOutput=False
    )
    b_p = nc.declare_dram_parameter("bvec", [1, NCOLS + 1], F32, isOutput=False)
    bas_p = nc.declare_dram_parameter("basis", [128, 4, N2], F32, isOutput=False)
    cam_p = nc.declare_dram_parameter("cam", [B, 12], F32, isOutput=False)
    eye_p = nc.declare_dram_parameter("eye", [B, B], F32, isOutput=False)
    out_p = nc.declare_dram_parameter("out", [B, 3, 2 * SL + 71 + 8], F32, isOutput=True)

    ar_in = nc.dram_tensor("ar_in", [B, NCOLS + 1], F32)
    ag_out = nc.dram_tensor("ag_out", [NCORES * B, NCOLS + 1], F32, addr_space="Shared")
    warm_in = nc.dram_tensor("warm_in", [1, 4], F32)
    warm_out = nc.dram_tensor("warm_out", [NCORES, 4], F32, addr_space="Shared")

    with tile.TileContext(nc) as tc:
        with (
            tc.tile_pool(name="consts", bufs=1) as consts,
            tc.tile_pool(name="xres", bufs=1) as xres,
            tc.tile_pool(name="latp", bufs=1) as latp,
            tc.tile_pool(name="geop", bufs=1) as geop,
            tc.tile_pool(name="planep", bufs=1) as planep,
            tc.tile_pool(name="dum", bufs=1, space="PSUM") as dum,
        ):
            # ---- const / prefetch loads ----
            eye_sb = consts.tile([B, B], F32)
            nc.scalar.dma_start(out=eye_sb, in_=eye_p[:, :])
            cam = consts.tile([B, 12], F32)
            nc.scalar.dma_start(out=cam, in_=cam_p[:, :])
            b_sb = consts.tile([1, NCOLS + 1], F32)
            nc.scalar.dma_start(out=b_sb, in_=b_p[:, :])
            ones1 = consts.tile([1, B], F32)
            nc.vector.memset(ones1, 1.0)
            halfpi = consts.tile([128, 1], F32)
            nc.vector.memset(halfpi, HALF_PI)
            lat = latp.tile([B, 416], F32)
            nc.vector.memset(lat, 0.0)

            # warm up the collective firmware path so the real gather below
            # does not pay first-op wakeup costs
            nc.gpsimd.collective_compute(
                "AllGather",
                ALU.bypass,
                replica_groups=[list(range(NCORES))],
                ins=[warm_in.ap().opt()],
                outs=[warm_out.ap().opt()],
            )

            # resident x shard (hi+lo), split so PE can start after part 0
            xts = []
            off = 0
            for pi, n in enumerate(XPARTS):
                xt = xres.tile([128, n, 2, B], BF16, name=f"xt{pi}")
                nc.gpsimd.dma_start(out=xt, in_=x_p[:, off:off + n, :, :])
                xts.append((off, n, xt))
                off += n
            # phase-2 basis block (prefetch; lands during phase 1)
            basis_sb = planep.tile([128, 4, N2], F32)
            nc.gpsimd.dma_start(out=basis_sb, in_=bas_p[:, :, :])

            def xap(k, hl):
                for off, n, xt in xts:
                    if k < off + n:
                        return xt[:, k - off, hl, :]
                raise IndexError(k)

            d1 = dum.tile([1, 1], F32)

            # ---------------- Phase 1: encoder GEMM (hi/lo bf16) ----------------
            with (
                tc.tile_pool(name="wts", bufs=6) as wts,
                tc.tile_pool(name="encp", bufs=1, space="PSUM") as encp,
            ):
                pe = encp.tile([B, NCOLS + 1], F32)
                nc.tensor.matmul(
                    d1, lhsT=xts[0][2][:, 0, 0, 0:1], rhs=xts[0][2][:, 0, 0, 0:1],
                    start=True, stop=True, skip_group_check=True,
                )
                for ci in range(NCH):
                    w_c = wts.tile([128, TPC, 2, NCOLS], BF16)
                    eng = nc.sync if ci % 2 == 0 else nc.scalar
                    eng.dma_start(out=w_c, in_=w_p[ci])
                    for t in range(TPC):
                        k = ci * TPC + t
                        nc.tensor.matmul(
                            pe[:, 0:NCOLS], lhsT=xap(k, 0), rhs=w_c[:, t, 0, :],
                            start=(k == 0), stop=False,
                        )
                        nc.tensor.matmul(
                            pe[:, 0:NCOLS], lhsT=xap(k, 0), rhs=w_c[:, t, 1, :],
                            start=False, stop=False,
                        )
                        nc.tensor.matmul(
                            pe[:, 0:NCOLS], lhsT=xap(k, 1), rhs=w_c[:, t, 0, :],
                            start=False, stop=False,
                        )
                # bias (scaled 1/8) + constant 1/8 lane in col 411, fp32
                nc.tensor.matmul(
                    d1, lhsT=b_sb[0:1, 0:1], rhs=b_sb[0:1, 0:1],
                    start=True, stop=True, skip_group_check=True,
                )
                nc.tensor.matmul(
                    pe, lhsT=ones1, rhs=b_sb, start=False, stop=True,
                )
                lat1 = latp.tile([B, NCOLS + 1], F32)
                nc.vector.tensor_copy(out=lat1, in_=pe)
                nc.sync.dma_start(out=ar_in[:, :], in_=lat1)

            nc.gpsimd.collective_compute(
                "AllGather",
                ALU.bypass,
                replica_groups=[list(range(NCORES))],
                ins=[ar_in.ap().opt()],
                outs=[ag_out.ap().opt()],
            )
            parts = latp.tile([B, NCORES, NCOLS + 1], F32)
            nc.sync.dma_start(
                out=parts, in_=ag_out.ap().rearrange("(c b) n -> b c n", b=B)
            )
            nc.vector.tensor_tensor(
                out=parts[:, 0:4, :], in0=parts[:, 0:4, :], in1=parts[:, 4:8, :],
                op=ALU.add,
            )
            nc.vector.tensor_tensor(
                out=parts[:, 0:2, :], in0=parts[:, 0:2, :], in1=parts[:, 2:4, :],
                op=ALU.add,
            )
            nc.vector.tensor_tensor(
                out=lat[:, 0:NCOLS + 1], in0=parts[:, 0, :], in1=parts[:, 1, :],
                op=ALU.add,
            )

            # ---------------- Phase 1.5: transpose shape params on PE ----------
            with tc.tile_pool(name="trps", bufs=1, space="PSUM") as trps:
                trp = trps.tile([128, 4, B], F32)
                nc.tensor.matmul(
                    d1, lhsT=eye_sb[0:1, 0:1], rhs=eye_sb[0:1, 0:1],
                    start=True, stop=True, skip_group_check=True,
                )
                for kt in range(3):
                    nc.tensor.matmul(
                        trp[:, kt, :], lhsT=lat[:, kt * 128:(kt + 1) * 128],
                        rhs=eye_sb, is_transpose=True,
                        start=True, stop=True, skip_group_check=True,
                    )
                nc.tensor.matmul(
                    trp[0:32, 3, :], lhsT=lat[:, 384:416],
                    rhs=eye_sb, is_transpose=True,
                    start=True, stop=True, skip_group_check=True,
                )
                spT = latp.tile([128, 4, B], F32)
                nc.scalar.copy(out=spT, in_=trp)

            # ---------------- Phase 2: blendshape GEMM (V-sharded) -------------
            vpre = planep.tile([B, N2], F32)
            NSPL2 = [(0, 512), (512, 512), (1024, 512), (1536, N2 - 1536)]
            with tc.tile_pool(name="p2ps", bufs=1, space="PSUM") as p2ps:
                pvs = [
                    p2ps.tile([B, n], F32, name=f"pv{j}", tag=f"pv{j}")
                    for j, (_, n) in enumerate(NSPL2)
                ]
                nc.tensor.matmul(
                    d1, lhsT=basis_sb[0:1, 0, 0:1], rhs=basis_sb[0:1, 0, 0:1],
                    start=True, stop=True, skip_group_check=True,
                )
                for j, (n0, n) in enumerate(NSPL2):
                    for kt in range(4):
                        rows = 128 if kt < 3 else 32
                        nc.tensor.matmul(
                            pvs[j],
                            lhsT=spT[0:rows, kt, :],
                            rhs=basis_sb[0:rows, kt, n0:n0 + n],
                            start=(kt == 0),
                            stop=(kt == 3),
                        )

                # eyeball rotation inputs (copies allowed on GpSimd)
                aa2 = geop.tile([128, 3], F32)
                nc.gpsimd.memset(aa2, 0.0)
                nc.gpsimd.tensor_copy(out=aa2[0:B, 0:2], in_=lat[:, P_LR:P_LR + 2])
                nc.sync.dma_start(out=aa2[B:128, 0:2], in_=lat[:, P_RR:P_RR + 2])

                # face rotation (DVE, overlaps the GEMM)
                g = Geo(nc, geop)
                Rf = axis_angle_R(nc, g, lat[:, P_ROT:P_ROT + 3], "f_", halfpi[:B, :])
                fs = g.t()
                nc.vector.tensor_scalar_add(
                    out=fs, in0=lat[:, P_SC:P_SC + 1], scalar1=1.0
                )
                Rs = geop.tile([B, 9], F32)
                nc.vector.tensor_scalar_mul(out=Rs, in0=Rf, scalar1=fs)

                for j, (n0, n) in enumerate([NSPL2[3], NSPL2[1], NSPL2[2], NSPL2[0]]):
                    nc.scalar.copy(out=vpre[:, n0:n0 + n], in_=pvs[NSPL2.index((n0, n))])

            # offsets: off_i = face_t_i - sum_l vms_l*Rs[l,i]
            off3 = geop.tile([B, 3], F32)
            for i in range(3):
                t = g.mul(vpre[:, 519:520], Rs[:, i:i + 1])
                t = g.mac(vpre[:, 520 + 519:520 + 520], Rs[:, 3 + i:4 + i], t)
                t = g.mac(vpre[:, 1040 + 519:1040 + 520], Rs[:, 6 + i:7 + i], t)
                nc.vector.tensor_tensor(
                    out=off3[:, i:i + 1], in0=lat[:, P_T + i:P_T + i + 1], in1=t,
                    op=ALU.subtract,
                )

            # rotate + translate all plane blocks
            rt = planep.tile([B, 3, PL], F32)
            for i in range(3):
                nc.vector.tensor_scalar(
                    out=rt[:, i, :], in0=vpre[:, 0:PL],
                    scalar1=Rs[:, i:i + 1], scalar2=off3[:, i:i + 1],
                    op0=ALU.mult, op1=ALU.add,
                )
                for l in (1, 2):
                    nc.vector.scalar_tensor_tensor(
                        out=rt[:, i, :], in0=vpre[:, l * PL:(l + 1) * PL],
                        scalar=Rs[:, 3 * l + i:3 * l + i + 1],
                        in1=rt[:, i, :],
                        op0=ALU.mult, op1=ALU.add,
                    )

            lc = [rt[:, i, SL + 68:SL + 69] for i in range(3)]
            rc = [rt[:, i, SL + 69:SL + 70] for i in range(3)]

            # projection of this core's vert slice (DVE)
            with tc.tile_pool(name="imgp", bufs=1) as imgp:
                img = imgp.tile([B, 3, SL], F32)
                for i in (2, 0, 1):  # z first (feeds the clamp chain)
                    nc.vector.tensor_scalar(
                        out=img[:, i, :], in0=rt[:, 0, 0:SL],
                        scalar1=cam[:, 4 * i:4 * i + 1],
                        scalar2=cam[:, 4 * i + 3:4 * i + 4],
                        op0=ALU.mult, op1=ALU.add,
                    )
                    for l in (1, 2):
                        nc.vector.scalar_tensor_tensor(
                            out=img[:, i, :], in0=rt[:, l, 0:SL],
                            scalar=cam[:, 4 * i + l:4 * i + l + 1], in1=img[:, i, :],
                            op0=ALU.mult, op1=ALU.add,
                        )
                az_ = imgp.tile([B, SL], F32)
                nc.scalar.activation(out=az_, in_=img[:, 2, :], func=ACTF.Abs)
                nc.vector.tensor_scalar_max(out=az_, in0=az_, scalar1=1e-3)
                sg = imgp.tile([B, SL], F32)
                nc.vector.tensor_scalar(
                    out=sg, in0=img[:, 2, :], scalar1=0.0, scalar2=None, op0=ALU.is_ge
                )
                nc.vector.tensor_scalar(
                    out=sg, in0=sg, scalar1=2.0, scalar2=1.0,
                    op0=ALU.mult, op1=ALU.subtract,
                )
                nc.vector.tensor_tensor(out=sg, in0=sg, in1=az_, op=ALU.mult)
                nc.vector.reciprocal(out=az_, in_=sg)
                nc.vector.tensor_tensor(
                    out=img[:, 0, :], in0=img[:, 0, :], in1=az_, op=ALU.mult
                )
                nc.vector.tensor_tensor(
                    out=img[:, 1, :], in0=img[:, 1, :], in1=az_, op=ALU.mult
                )

                # eyeball rotations (DVE tail; only gates the Cramer solve)
                g2 = Geo(nc, geop, rows=128)
                R2 = axis_angle_R(nc, g2, aa2, "e_", halfpi)
                gz = geop.tile([128, 3], F32)
                nc.vector.tensor_scalar_mul(out=gz, in0=R2[:, 6:9], scalar1=GAZE_DIR)
                rg64 = geop.tile([B, 3], F32)
                nc.sync.dma_start(out=rg64, in_=gz[B:128, :])
                lg = [gz[0:B, i:i + 1] for i in range(3)]
                rg = [rg64[:, i:i + 1] for i in range(3)]

                # vert + img outputs can ship while the tail is computed
                nc.sync.dma_start(out=out_p[:, :, 0:SL], in_=rt[:, :, 0:SL])
                nc.scalar.dma_start(out=out_p[:, :, SL:2 * SL], in_=img)
                nc.sync.dma_start(
                    out=out_p[:, :, 2 * SL:2 * SL + 71], in_=rt[:, :, SL:SL + 71]
                )

                # tail block ge[:, i, j]: gp_l gp_r gp_mid far_l far_r lg rg dist
                ge = geop.tile([B, 3, 8], F32)
                for i in range(3):
                    # independent pieces off the DVE critical chain
                    nc.vector.scalar_tensor_tensor(
                        out=ge[:, i, 3:4], in0=lg[i], scalar=1000.0,
                        in1=lc[i], op0=ALU.mult, op1=ALU.add,
                    )
                    nc.vector.scalar_tensor_tensor(
                        out=ge[:, i, 4:5], in0=rg[i], scalar=1000.0,
                        in1=rc[i], op0=ALU.mult, op1=ALU.add,
                    )
                    nc.gpsimd.tensor_copy(out=ge[:, i, 5:6], in_=lg[i])
                    nc.gpsimd.tensor_copy(out=ge[:, i, 6:7], in_=rg[i])

                # gaze intersection (Cramer, DVE)
                d = [g.sub(rc[i], lc[i]) for i in range(3)]
                c1 = []
                for i in range(3):
                    o = g.t()
                    nc.vector.tensor_scalar_mul(out=o, in0=rg[i], scalar1=-1.0)
                    c1.append(o)
                c2 = list(g.cross3(*rg, *lg))
                w = g.cross3(*c1, *c2)
                det = g.dot3(*lg, *w)
                num0 = g.dot3(*d, *w)
                w2 = g.cross3(*d, *c2)
                num1 = g.dot3(*lg, *w2)
                rdet = g.t()
                nc.vector.reciprocal(out=rdet, in_=det)
                sol0 = g.mul(num0, rdet)
                sol1 = g.mul(num1, rdet)

                gpl = geop.tile([B, 3], F32)
                gpr = geop.tile([B, 3], F32)
                for i in range(3):
                    nc.vector.scalar_tensor_tensor(
                        out=gpl[:, i:i + 1], in0=lg[i], scalar=sol0,
                        in1=lc[i], op0=ALU.mult, op1=ALU.add,
                    )
                    nc.vector.scalar_tensor_tensor(
                        out=gpr[:, i:i + 1], in0=rg[i], scalar=sol1,
                        in1=rc[i], op0=ALU.mult, op1=ALU.add,
                    )
                    nc.vector.tensor_copy(out=ge[:, i, 0:1], in_=gpl[:, i:i + 1])
                    nc.vector.tensor_copy(out=ge[:, i, 1:2], in_=gpr[:, i:i + 1])
                    o = g.add(gpl[:, i:i + 1], gpr[:, i:i + 1])
                    nc.vector.tensor_scalar_mul(out=ge[:, i, 2:3], in0=o, scalar1=0.5)
                dff = geop.tile([B, 3], F32)
                nc.vector.tensor_tensor(out=dff, in0=gpl, in1=gpr, op=ALU.subtract)
                nc.vector.tensor_tensor(out=dff, in0=dff, in1=dff, op=ALU.mult)
                d2 = g.t()
                nc.vector.tensor_reduce(out=d2, in_=dff, axis=AX.X, op=ALU.add)
                dist = g.t()
                nc.scalar.activation(out=dist, in_=d2, func=ACTF.Sqrt)
                for i in range(3):
                    nc.scalar.copy(out=ge[:, i, 7:8], in_=dist)

                nc.scalar.dma_start(out=out_p[:, :, 2 * SL + 71:2 * SL + 79], in_=ge)
    _legalize_waits(nc)
    return nc


def _prep(inputs):
    f32 = np.float32
    x = np.ascontiguousarray(inputs["x"].reshape(B, DIN), dtype=f32)
    W = np.asarray(inputs["enc_W"], dtype=f32)
    Wp = np.concatenate([W[:, :400], W[:, 545:556]], axis=1)  # [DIN, 411]
    enc_b = np.asarray(inputs["enc_b"], dtype=f32)
    bp = np.concatenate([enc_b[:400], enc_b[545:556]])
    bvec = np.concatenate(
        [bp / NCORES, np.array([1.0 / NCORES], f32)]
    ).reshape(1, NCOLS + 1).astype(f32)
    tmpl = np.asarray(inputs["v_template"], dtype=f32)  # [V, 3]
    basis = np.asarray(inputs["shape_basis"], dtype=f32)  # [400, V, 3]
    cam = np.ascontiguousarray(
        np.asarray(inputs["camera_parameters"], dtype=f32).reshape(B, 12)
    )
    lm = np.asarray(inputs["landmarks"])
    mlm = np.asarray(inputs["masked_landmarks"])
    fmask = np.asarray(inputs["face_mask"])
    lmask = np.asarray(inputs["left_eyeball_mask"])
    rmask = np.asarray(inputs["right_eyeball_mask"])
    fl_idx = fmask[mlm]  # verts behind the 68 output landmarks
    idx4 = lm[np.array([19, 22, 25, 28])]
    idx2 = lm[np.array([14, 18])]

    # synthetic extra columns [400, 72, 3] / [72, 3]
    ex_b = np.concatenate([
        basis[:, fl_idx, :],
        basis[:, lmask, :].mean(axis=1, keepdims=True),
        basis[:, rmask, :].mean(axis=1, keepdims=True),
        (basis[:, idx4, :].mean(axis=1, keepdims=True)
         + basis[:, idx2, :].mean(axis=1, keepdims=True)) / 2.0,
        basis.mean(axis=1, keepdims=True),
    ], axis=1)
    ex_t = np.concatenate([
        tmpl[fl_idx],
        tmpl[lmask].mean(axis=0, keepdims=True),
        tmpl[rmask].mean(axis=0, keepdims=True),
        (tmpl[idx4].mean(axis=0, keepdims=True)
         + tmpl[idx2].mean(axis=0, keepdims=True)) / 2.0,
        tmpl.mean(axis=0, keepdims=True),
    ], axis=0)

    eye = np.eye(B, dtype=f32)
    in_maps = []
    for c in range(NCORES):
        k0 = c * KSH
        xs = x[:, k0:k0 + KSH].T  # [KSH, B] f32
        xh = xs.astype(BF)
        xl = (xs - xh.astype(f32)).astype(BF)
        xw = np.ascontiguousarray(
            np.stack([
                xh.reshape(KT, 128, B).transpose(1, 0, 2),
                xl.reshape(KT, 128, B).transpose(1, 0, 2),
            ], axis=2)
        )  # [128, KT, 2, B] bf16
        ws = Wp[k0:k0 + KSH]  # [KSH, 411] f32
        wh = ws.astype(BF)
        wl = (ws - wh.astype(f32)).astype(BF)
        wch = np.ascontiguousarray(
            np.stack([
                wh.reshape(NCH, TPC, 128, NCOLS).transpose(0, 2, 1, 3),
                wl.reshape(NCH, TPC, 128, NCOLS).transpose(0, 2, 1, 3),
            ], axis=3)
        )  # [NCH, 128, TPC, 2, 411] bf16

        lo = c * SL
        verts = fmask[lo:min(lo + SL, VM)]
        nsl = len(verts)
        blk = np.zeros((400, N2), f32)
        trow = np.zeros(N2, f32)
        for l in range(3):
            blk[:, l * PL:l * PL + nsl] = basis[:, verts, l]
            blk[:, l * PL + SL:l * PL + SL + 72] = ex_b[:, :, l]
            trow[l * PL:l * PL + nsl] = tmpl[verts, l]
            trow[l * PL + SL:l * PL + SL + 72] = ex_t[:, l]
        bh = np.zeros((128, 4, N2), f32)
        for kt in range(3):
            bh[:, kt, :] = blk[kt * 128:(kt + 1) * 128]
        bh[0:16, 3, :] = blk[384:400]
        bh[27, 3, :] = trow  # coefficient = exact 1.0 from AR col 411
        in_maps.append({
            "xw": xw,
            "wch": wch,
            "bvec": bvec,
            "basis": np.ascontiguousarray(bh),
            "cam": cam,
            "eye": eye,
        })
    return in_maps


def _run(inputs, trace=False):
    in_maps = _prep(inputs)
    nc = build_graph()
    res = run_bass_kernel_spmd(
        nc, in_maps, core_ids=list(range(NCORES)), trace=trace
    )
    full = np.empty((B, 3, NOUT), np.float32)
    for c in range(NCORES):
        r = res.results[c]["out"]  # [B, 3, 975]
        lo = c * SL
        w = min(SL, VM - lo)
        full[:, :, lo:lo + w] = r[:, :, 0:w]
        full[:, :, VM + lo:VM + lo + w] = r[:, :, SL:SL + w]
    r0 = res.results[0]["out"]
    full[:, :, 2 * VM:NOUT] = r0[:, :, 2 * SL:2 * SL + 79]
    return np.ascontiguousarray(full.transpose(0, 2, 1)), res


def kernel(**inputs):
    out, _ = _run(inputs, trace=False)
    return out
